# revision 1
# baseline (speedup 1.0000x reference)
"""CrossAttention kernel for 8 TRN2 NeuronCores.

Sharding: 8 cores = 4 batches x 2 query-halves (zero communication).
Each core computes all 16 heads for its 1024 queries.

v5 structure:
- x/ctx pre-transposed AND all projection operands pre-cast to fp8e4m3
  on the host: halves the HBM load traffic (startup was bandwidth
  bound) and enables DoubleRow fp8 matmuls (~1.4x) for the q/k/v/out
  projections. Scores and AV matmuls stay bf16 for accuracy.
- loads chunked in consumer order across the sync/scalar/gpsimd rings
  so the first k-projection starts ~6us in.
- attention emits the AV matmuls lagged one kb iteration behind the
  scores so the next head pair never blocks on the AV-PSUM evacuation.
- softmax denominators: the 4 PSUM ones-rows are copied to partitions
  {0,32,64,96} of one tile, one batched reciprocal_approx_fast serves
  the whole head pair, and 1/den is broadcast across partitions with
  K=1 matmuls into PSUM (no DRAM round trip), then one DVE multiply
  normalizes and the result is stored as the fp8 out-proj operand.
- output stored bf16 (upcast on host) to halve the store traffic.
"""

import sys

for _p in ("/opt/trn_rl_repo", "/root/.axon_site/_ro/trn_rl_repo"):
    if _p not in sys.path:
        sys.path.append(_p)

import numpy as np

import concourse.bass as bass
import concourse.tile as tile
from concourse import bacc, mybir
from concourse.bass_utils import run_bass_kernel_spmd

F32 = mybir.dt.float32
BF16 = mybir.dt.bfloat16
FP8 = mybir.dt.float8e4
DR = mybir.MatmulPerfMode.DoubleRow
EXP = mybir.ActivationFunctionType.Exp
MULT = mybir.AluOpType.mult

P = 128
B, NQ_FULL, DQ = 4, 2048, 1024
NK, DC = 1024, 768
H, DH = 16, 64
INNER = H * DH  # 1024
NT = 1024  # local queries per core
N_CORES = 8

KQ = DQ // P      # 8
KC = DC // P      # 6
KI = INNER // P   # 8
TB = NT // P      # 8
KB = NK // P      # 8
HP = H // 2       # 8 head pairs
SCALE = 1.0 / np.sqrt(DH)


def build(dbg=False):
    nc = bacc.Bacc("TRN2", target_bir_lowering=False, debug=False,
                   enable_asserts=False, num_devices=N_CORES)

    xT_d = nc.dram_tensor("xT", [DQ, NT], BF16, kind="ExternalInput")
    cT_d = nc.dram_tensor("cT", [DC, NK], BF16, kind="ExternalInput")
    wq_d = nc.dram_tensor("wq", [DQ, INNER], BF16, kind="ExternalInput")
    wk_d = nc.dram_tensor("wk", [DC, INNER], BF16, kind="ExternalInput")
    wv_d = nc.dram_tensor("wv", [DC, INNER], BF16, kind="ExternalInput")
    wo_d = nc.dram_tensor("wo", [INNER, DQ], BF16, kind="ExternalInput")
    bo_d = nc.dram_tensor("bo", [DQ], BF16, kind="ExternalInput")
    out_d = nc.dram_tensor("out", [NT, DQ], BF16, kind="ExternalOutput")
    if dbg:
        dqT = nc.dram_tensor("dqT", [P, KI, NT], F32, kind="ExternalOutput")
        dkT = nc.dram_tensor("dkT", [P, KI, NK], F32, kind="ExternalOutput")
        dvA = nc.dram_tensor("dvA", [P, KB, H, DH + 1], F32,
                             kind="ExternalOutput")
        dav = nc.dram_tensor("dav", [P, HP, NT], F32, kind="ExternalOutput")
        drec = nc.dram_tensor("drec", [4, HP, 512], F32,
                              kind="ExternalOutput")
        dattnT = nc.dram_tensor("dattnT", [P, KI, NT], F32,
                                kind="ExternalOutput")

    with tile.TileContext(nc) as tc:
        with (
            tc.tile_pool(name="persist", bufs=1) as persist,
            tc.tile_pool(name="psA", bufs=2, space="PSUM") as psA,
            tc.tile_pool(name="psV", bufs=4, space="PSUM") as psV,
            tc.tile_pool(name="expp", bufs=10) as expp,
            tc.tile_pool(name="avp", bufs=2) as avp,
            tc.tile_pool(name="recp", bufs=2) as recp,
            tc.tile_pool(name="outp", bufs=2) as outp,
        ):
            # persistent SBUF tensors
            xT = persist.tile([P, KQ, NT], BF16)       # [dq, q]
            cT = persist.tile([P, KC, NK], BF16)       # [dc, kpos]
            wq_b = persist.tile([P, KQ, INNER], BF16)
            wk_b = persist.tile([P, KC, INNER], BF16)
            wv_b = persist.tile([P, KC, INNER], BF16)
            wo_b = persist.tile([P, KI, DQ], BF16)
            bo_sb = persist.tile([1, DQ], BF16)
            ones_b = persist.tile([1, P], BF16)
            ones4 = persist.tile([97, DH], BF16)      # K=1 bcast lhsT rows
            qT = persist.tile([P, KI, NT], BF16)      # [inner, q]
            kT = persist.tile([P, KI, NK], BF16)      # [inner, kpos]
            vA = persist.tile([P, KB, H, DH + 1], BF16)  # [kpos,(h, d|1)]
            attnT = persist.tile([P, KI, NT], BF16)    # normalized attn out

            # ---------------- input loads (consumer order) --------------
            # scalar's sequencer is kept DMA-free: its HWDGE descriptor
            # generation would queue ahead of the ACTIVATE stream
            cT3 = cT_d.ap().rearrange("(o p) m -> p o m", p=P)
            xT3 = xT_d.ap().rearrange("(o p) m -> p o m", p=P)
            wk4 = wk_d.ap().rearrange("(o p) m -> p o m", p=P)
            wq4 = wq_d.ap().rearrange("(o p) m -> p o m", p=P)
            wv4 = wv_d.ap().rearrange("(o p) m -> p o m", p=P)
            wo4 = wo_d.ap().rearrange("(o p) m -> p o m", p=P)
            nc.sync.dma_start(cT[:], cT3)
            nc.sync.dma_start(xT[:], xT3)
            nc.gpsimd.dma_start(wk_b[:], wk4)
            nc.gpsimd.dma_start(wq_b[:], wq4)
            nc.gpsimd.dma_start(wv_b[:], wv4)
            nc.gpsimd.dma_start(wo_b[:], wo4)
            nc.gpsimd.dma_start(bo_sb[:], bo_d.ap()[None, :])
            nc.gpsimd.memset(vA[:, :, :, DH:DH + 1], 1.0)
            nc.gpsimd.memset(ones_b[:], 1.0)
            nc.gpsimd.memset(ones4[:], 1.0)

            # ---------------- projection helpers (fp8 DoubleRow) --------
            def kproj(ko):
                ps = psA.tile([P, NT], F32, tag="big", name=f"kp{ko}")
                for n0 in (0, 512):
                    for kc in range(KC):
                        nc.tensor.matmul(
                            ps[:, n0:n0 + 512],
                            wk_b[:, kc, ko * P:(ko + 1) * P],
                            cT[:, kc, n0:n0 + 512],
                            start=(kc == 0), stop=(kc == KC - 1))
                nc.vector.tensor_copy(kT[:, ko, :], ps[:])

            def qproj(ko):
                ps = psA.tile([P, NT], F32, tag="big", name=f"qp{ko}")
                for n0 in (0, 512):
                    for kc in range(KQ):
                        nc.tensor.matmul(
                            ps[:, n0:n0 + 512],
                            wq_b[:, kc, ko * P:(ko + 1) * P],
                            xT[:, kc, n0:n0 + 512],
                            start=(kc == 0), stop=(kc == KQ - 1))
                nc.vector.tensor_copy(qT[:, ko, :], ps[:])

            def vproj(mt, half):
                n0 = half * 512
                ps = psA.tile([P, NT], F32, tag="big", name=f"vp{mt}_{half}")
                for kc in range(KC):
                    nc.tensor.matmul(
                        ps[:, 0:512],
                        cT[:, kc, mt * P:(mt + 1) * P],
                        wv_b[:, kc, n0:n0 + 512],
                        start=(kc == 0), stop=(kc == KC - 1))
                h0 = half * 8
                nc.vector.tensor_copy(
                    vA[:, mt, h0:h0 + 8, 0:DH],
                    ps[:, 0:512].rearrange("p (h d) -> p h d", d=DH))

            # ---------------- attention ----------------
            def attn_head_pair(hp, extra_pe=None):
                h0, h1 = 2 * hp, 2 * hp + 1
                psvs = {h: [psV.tile([DH + 1, 512], F32, tag="av",
                                     name=f"psv{h}_{n}") for n in (0, 1)]
                        for h in (h0, h1)}
                ets = {}
                for kb in range(KB):
                    pss = {h: psA.tile([P, NT], F32, tag="big",
                                       name=f"sc{h}_{kb}") for h in (h0, h1)}
                    # n0-major so the two heads' K=64 matmuls pair up on
                    # different PE row groups and run concurrently
                    for n0 in (0, 512):
                        for h in (h0, h1):
                            base = (h % 2) * DH
                            nc.tensor.matmul(
                                pss[h][:, n0:n0 + 512],
                                kT[base:base + DH, hp, kb * P:(kb + 1) * P],
                                qT[base:base + DH, hp, n0:n0 + 512],
                                start=True, stop=True)
                    if extra_pe:
                        for fn in extra_pe.pop(0):
                            fn()
                    for h in (h0, h1):
                        et = expp.tile([P, NT], BF16, tag="exp")
                        nc.scalar.activation(et[:], pss[h][:], EXP,
                                             scale=float(SCALE))
                        ets[(h, kb)] = et
                    # AV lags one kb so the PE never blocks on psV slots
                    # that the previous pair is still evacuating
                    if kb > 0:
                        emit_av(hp, psvs, ets, kb - 1)
                emit_av(hp, psvs, ets, KB - 1)

                # evacuate AV PSUM + denominator rows (DVE work that can
                # drain while the next head pair's scores run)
                av_pair = avp.tile([P, NT], F32, tag="avsb")
                den4 = recp.tile([97, 512], F32, tag="den")
                rec4 = recp.tile([97, 512], F32, tag="rec")
                rec4b = recp.tile([97, 512], BF16, tag="recb")
                for i, h in enumerate((h0, h1)):
                    for ni, n0 in enumerate((0, 512)):
                        nc.vector.tensor_copy(
                            av_pair[i * DH:(i + 1) * DH, n0:n0 + 512],
                            psvs[h][ni][0:DH, :])
                        pr = i * 64 + ni * 32
                        nc.vector.tensor_copy(den4[pr:pr + 1, :],
                                              psvs[h][ni][DH:DH + 1, :])
                nc.vector.reciprocal_approx_fast(rec4[:], den4[:])
                nc.vector.tensor_copy(rec4b[:], rec4[:])

                def finish():
                    # broadcast 1/den across partitions via K=1 matmuls,
                    # then one multiply normalizes the pair. Deferred into
                    # the next head pair's loop so the PE never waits on
                    # the DVE chain above.
                    rb_ps = psA.tile([P, NT], F32, tag="big", name=f"rb{hp}")
                    for i in (0, 1):
                        for ni, n0 in enumerate((0, 512)):
                            pr = i * 64 + ni * 32
                            nc.tensor.matmul(
                                rb_ps[i * DH:(i + 1) * DH, n0:n0 + 512],
                                ones4[pr:pr + 1, :],
                                rec4b[pr:pr + 1, :],
                                start=True, stop=True,
                                tile_position=(pr, i * DH))
                    nc.vector.tensor_tensor(attnT[:, hp, :], av_pair[:],
                                            rb_ps[:], MULT)
                    if dbg:
                        nc.gpsimd.dma_start(dav.ap()[:, hp, :], av_pair[:])

                return finish

            def emit_av(hp, psvs, ets, kb):
                h0, h1 = 2 * hp, 2 * hp + 1
                for h in (h0, h1):
                    for ni, n0 in enumerate((0, 512)):
                        nc.tensor.matmul(
                            psvs[h][ni][:],
                            vA[:, kb, h, :],
                            ets[(h, kb)][:, n0:n0 + 512],
                            start=(kb == 0), stop=(kb == KB - 1))

            # ---------------- schedule ----------------
            kproj(0)
            qproj(0)
            pending = None
            for hp in range(HP):
                extra = [[] for _ in range(KB)]
                if pending is not None:
                    extra[1].append(pending)
                if hp == 0:
                    for mt in range(KB):
                        extra[mt].append(lambda mt=mt: vproj(mt, 0))
                if hp < HP - 1:
                    extra[0].append(lambda ko=hp + 1: kproj(ko))
                    extra[2].append(lambda ko=hp + 1: qproj(ko))
                if hp in (1, 2):
                    for j in range(4):
                        mt = (hp - 1) * 4 + j
                        extra[4 + j].append(lambda mt=mt: vproj(mt, 1))
                pending = attn_head_pair(hp, extra)
            pending()

            if dbg:
                nc.gpsimd.dma_start(dqT.ap(), qT[:])
                nc.gpsimd.dma_start(dkT.ap(), kT[:])
                nc.gpsimd.dma_start(dvA.ap(), vA[:])
                nc.gpsimd.dma_start(dattnT.ap(), attnT[:])

            # ---------------- out projection (fp8 DoubleRow) ------------
            out3 = out_d.ap().rearrange("(t p) d -> p t d", p=P)

            def out_mms(ps, mt, kc_range, start):
                for n0 in (0, 512):
                    for kc in kc_range:
                        nc.tensor.matmul(
                            ps[:, n0:n0 + 512],
                            attnT[:, kc, mt * P:(mt + 1) * P],
                            wo_b[:, kc, n0:n0 + 512],
                            start=(start and kc == kc_range[0]), stop=False)

            def out_finish(ps, mt):
                for n0 in (0, 512):
                    nc.tensor.matmul(
                        ps[:, n0:n0 + 512],
                        attnT[:, KI - 1, mt * P:(mt + 1) * P],
                        wo_b[:, KI - 1, n0:n0 + 512],
                        start=False, stop=False)
                    nc.tensor.matmul(
                        ps[:, n0:n0 + 512],
                        ones_b[0:1, :],
                        bo_sb[0:1, n0:n0 + 512],
                        start=False, stop=True)
                ot = outp.tile([P, DQ], BF16, tag="out")
                nc.vector.tensor_copy(ot[:], ps[:])
                eng = nc.sync if mt % 2 == 0 else nc.scalar
                eng.dma_start(out3[:, mt], ot[:])

            # first two tiles: prefetch the head-0..13 contributions while
            # the last head pair is still normalizing
            ps0 = psA.tile([P, NT], F32, tag="big", name="op0")
            out_mms(ps0, 0, list(range(KI - 1)), True)
            ps1 = psA.tile([P, NT], F32, tag="big", name="op1")
            out_mms(ps1, 1, list(range(KI - 1)), True)
            out_finish(ps0, 0)
            out_finish(ps1, 1)
            for mt in range(2, TB):
                ps = psA.tile([P, NT], F32, tag="big", name=f"op{mt}")
                out_mms(ps, mt, list(range(KI - 1)), True)
                out_finish(ps, mt)

    nc.compile()
    return nc


_NC_CACHE = None


def _make_in_maps(inputs):
    import ml_dtypes
    bf = ml_dtypes.bfloat16
    x = np.asarray(inputs["x"], dtype=np.float32).astype(bf)
    context = np.asarray(inputs["context"], dtype=np.float32).astype(bf)
    shared = {
        "wq": np.ascontiguousarray(np.asarray(inputs["Wq"], np.float32).astype(bf)),
        "wk": np.ascontiguousarray(np.asarray(inputs["Wk"], np.float32).astype(bf)),
        "wv": np.ascontiguousarray(np.asarray(inputs["Wv"], np.float32).astype(bf)),
        "wo": np.ascontiguousarray(np.asarray(inputs["Wo"], np.float32).astype(bf)),
        "bo": np.ascontiguousarray(np.asarray(inputs["bo"], np.float32).astype(bf)),
    }
    in_maps = []
    for c in range(N_CORES):
        b, s = divmod(c, 2)
        in_maps.append({
            "xT": np.ascontiguousarray(x[b, s * NT:(s + 1) * NT, :].T),
            "cT": np.ascontiguousarray(context[b].T),
            **shared,
        })
    return in_maps


def kernel(x, context, Wq, Wk, Wv, Wo, bo):
    global _NC_CACHE
    if _NC_CACHE is None:
        _NC_CACHE = build()
    nc = _NC_CACHE

    in_maps = _make_in_maps(dict(x=x, context=context, Wq=Wq, Wk=Wk, Wv=Wv,
                                 Wo=Wo, bo=bo))
    res = run_bass_kernel_spmd(nc, in_maps, core_ids=list(range(N_CORES)))
    out = np.empty((B, NQ_FULL, DQ), dtype=np.float32)
    for c in range(N_CORES):
        b, s = divmod(c, 2)
        out[b, s * NT:(s + 1) * NT, :] = res.results[c]["out"].astype(
            np.float32)
    return out



# revision 7
# speedup vs baseline: 1.1326x; 1.1326x over previous
"""CrossAttention kernel for 8 TRN2 NeuronCores.

Sharding: 8 cores = 4 batches x 2 query-halves (zero communication).
Each core computes all 16 heads for its 1024 queries.

v7 structure (cost model: matmul time = out-free-cols x 0.417ns, K/M free):
- AV computed in [q, d] orientation (lhsT = exp-scores tile, rhs = V):
  66.5k streamed columns instead of 131k for the [d, q] orientation.
  A ones-column appended to V gives the softmax denominator as column 64
  of each AV psum region -- no separate denominator matmuls.
- PSUM accumulations within one bank must be sequential (interleaving
  corrupts earlier regions), so heads are processed one at a time: head
  h's scores+exp stream in block h while head h-1's AV regions run
  region-major (kb innermost), packed 7-per-bank into rolling psum
  banks (3 rotating) one block behind.
- normalization is a per-partition DVE reciprocal + tensor_scalar
  multiply (q on partitions); normalized [q, 128] pair tiles go back to
  [inner, q] via DMA xbar transposes (zero PE cost).
- bias folded into the out evacuation via a precomputed broadcast tile.
- weight/x loads host-packed so each kproj/qproj ko-chunk is one
  contiguous-descriptor DMA, issued in consumer order.
"""

import sys

for _p in ("/opt/trn_rl_repo", "/root/.axon_site/_ro/trn_rl_repo"):
    if _p not in sys.path:
        sys.path.append(_p)

import numpy as np

import concourse.bass as bass
import concourse.tile as tile
from concourse import bacc, mybir
from concourse.bass_utils import run_bass_kernel_spmd

F32 = mybir.dt.float32
BF16 = mybir.dt.bfloat16
EXP = mybir.ActivationFunctionType.Exp
ADD = mybir.AluOpType.add

P = 128
B, NQ_FULL, DQ = 4, 2048, 1024
NK, DC = 1024, 768
H, DH = 16, 64
INNER = H * DH  # 1024
NT = 1024  # local queries per core
N_CORES = 8

KQ = DQ // P      # 8
KC = DC // P      # 6
KI = INNER // P   # 8
TB = NT // P      # 8 query tiles
KB = NK // P      # 8 kpos chunks
QB = NT // P      # 8 q-blocks for AV
HP = H // 2       # 8 head pairs
SCALE = 1.0 / np.sqrt(DH)


def build(dbg=False):
    nc = bacc.Bacc("TRN2", target_bir_lowering=False, debug=False,
                   enable_asserts=False, num_devices=N_CORES)

    cT_d = nc.dram_tensor("cT", [DC, NK], BF16, kind="ExternalInput")
    xpk_d = nc.dram_tensor("xpk", [P, 2, KQ, 512], BF16, kind="ExternalInput")
    wqpk_d = nc.dram_tensor("wqpk", [P, KI, KQ, P], BF16,
                            kind="ExternalInput")
    wkpk_d = nc.dram_tensor("wkpk", [P, KI, KC, P], BF16,
                            kind="ExternalInput")
    wvpk_d = nc.dram_tensor("wvpk", [P, 2, KC, 512], BF16,
                            kind="ExternalInput")
    wo_d = nc.dram_tensor("wo", [INNER, DQ], BF16, kind="ExternalInput")
    bo_d = nc.dram_tensor("bo", [DQ], BF16, kind="ExternalInput")
    out_d = nc.dram_tensor("out", [NT, DQ], BF16, kind="ExternalOutput")
    if dbg:
        dqT = nc.dram_tensor("dqT", [P, KI, NT], F32, kind="ExternalOutput")
        dkT = nc.dram_tensor("dkT", [P, KI, NK], F32, kind="ExternalOutput")
        dvA = nc.dram_tensor("dvA", [P, KB, H, DH + 1], F32,
                             kind="ExternalOutput")
        dattnT = nc.dram_tensor("dattnT", [P, KI, NT], F32,
                                kind="ExternalOutput")

    with tile.TileContext(nc) as tc:
        with (
            tc.tile_pool(name="persist", bufs=1) as persist,
            tc.tile_pool(name="psS", bufs=2, space="PSUM") as psS,
            tc.tile_pool(name="psAV", bufs=3, space="PSUM") as psAV,
            tc.tile_pool(name="psPJ", bufs=1, space="PSUM") as psPJ,
            tc.tile_pool(name="etp", bufs=18) as etp,
            tc.tile_pool(name="aqp", bufs=20) as aqp,
            tc.tile_pool(name="recp", bufs=8) as recp,
            tc.tile_pool(name="outp", bufs=2) as outp,
        ):
            # persistent SBUF tensors
            cT = persist.tile([P, KC, NK], BF16)          # [dc, kpos]
            xT = persist.tile([P, 2, KQ, 512], BF16)      # [dq, (half,kc,q)]
            wq_b = persist.tile([P, KI, KQ, P], BF16)
            wk_b = persist.tile([P, KI, KC, P], BF16)
            wv_b = persist.tile([P, 2, KC, 512], BF16)
            wo_b = persist.tile([P, KI, DQ], BF16)
            bo_sb = persist.tile([1, DQ], BF16)
            ones_b = persist.tile([1, P], BF16)
            bias_b = persist.tile([P, DQ], BF16)          # bo bcast over parts
            qT = persist.tile([P, KI, NT], BF16)          # [inner, q]
            kT = persist.tile([P, KI, NK], BF16)          # [inner, kpos]
            vA = persist.tile([P, KB, H, DH + 1], BF16)   # [kpos,(h, d|1)]
            attnT = persist.tile([P, KI, NT], BF16)       # normalized attn^T

            # ---------------- input loads (consumer order) --------------
            cT3 = cT_d.ap().rearrange("(o p) m -> p o m", p=P)
            wo4 = wo_d.ap().rearrange("(o p) m -> p o m", p=P)
            nc.sync.dma_start(cT[:], cT3)
            nc.sync.dma_start(xT[:, 0], xpk_d.ap()[:, 0])
            nc.sync.dma_start(xT[:, 1], xpk_d.ap()[:, 1])
            nc.sync.dma_start(wo_b[:], wo4)
            nc.sync.dma_start(bo_sb[:], bo_d.ap()[None, :])
            # wq ko-chunks on the scalar ring (idle until first exp)
            for ko in range(KI):
                nc.scalar.dma_start(wq_b[:, ko], wqpk_d.ap()[:, ko])
            # wk ko-chunks + wv halves on the gpsimd ring
            nc.gpsimd.memset(vA[:, :, :, DH:DH + 1], 1.0)
            nc.gpsimd.memset(ones_b[:], 1.0)
            for ko in (0, 1, 2):
                nc.gpsimd.dma_start(wk_b[:, ko], wkpk_d.ap()[:, ko])
            nc.gpsimd.dma_start(wv_b[:, 0], wvpk_d.ap()[:, 0])
            nc.gpsimd.dma_start(wk_b[:, 3], wkpk_d.ap()[:, 3])
            nc.gpsimd.dma_start(wv_b[:, 1], wvpk_d.ap()[:, 1])
            for ko in (4, 5, 6, 7):
                nc.gpsimd.dma_start(wk_b[:, ko], wkpk_d.ap()[:, ko])

            # ---------------- projection helpers ------------------------
            # startup projections get full psS tiles; the steady-state ones
            # run through the single-bank psPJ pool in 512-col halves.
            def kproj0():
                ps = psS.tile([P, NT], F32, tag="big", name="kp0")
                for n0 in (0, 512):
                    for kc in range(KC):
                        nc.tensor.matmul(
                            ps[:, n0:n0 + 512],
                            wk_b[:, 0, kc, :],
                            cT[:, kc, n0:n0 + 512],
                            start=(kc == 0), stop=(kc == KC - 1))
                nc.vector.tensor_copy(kT[:, 0, :], ps[:])

            def qproj0():
                ps = psS.tile([P, NT], F32, tag="big", name="qp0")
                # bias_b broadcast rides in this psum tile first
                for n0 in (0, 512):
                    nc.tensor.matmul(ps[:, n0:n0 + 512], ones_b[0:1, :],
                                     bo_sb[0:1, n0:n0 + 512],
                                     start=True, stop=True)
                nc.vector.tensor_copy(bias_b[:], ps[:])
                for hf in (0, 1):
                    n0 = hf * 512
                    for kc in range(KQ):
                        nc.tensor.matmul(
                            ps[:, n0:n0 + 512],
                            wq_b[:, 0, kc, :],
                            xT[:, hf, kc, :],
                            start=(kc == 0), stop=(kc == KQ - 1))
                nc.vector.tensor_copy(qT[:, 0, :], ps[:])

            def kproj_half(ko, hf):
                n0 = hf * 512
                ps = psPJ.tile([P, 512], F32, tag="pj", name=f"kp{ko}_{hf}")
                for kc in range(KC):
                    nc.tensor.matmul(
                        ps[:], wk_b[:, ko, kc, :], cT[:, kc, n0:n0 + 512],
                        start=(kc == 0), stop=(kc == KC - 1))
                nc.vector.tensor_copy(kT[:, ko, n0:n0 + 512], ps[:])

            def qproj_half(ko, hf):
                n0 = hf * 512
                ps = psPJ.tile([P, 512], F32, tag="pj", name=f"qp{ko}_{hf}")
                for kc in range(KQ):
                    nc.tensor.matmul(
                        ps[:], wq_b[:, ko, kc, :], xT[:, hf, kc, :],
                        start=(kc == 0), stop=(kc == KQ - 1))
                nc.vector.tensor_copy(qT[:, ko, n0:n0 + 512], ps[:])

            def vproj(mt, half):
                ps = psPJ.tile([P, 512], F32, tag="pj", name=f"vp{mt}_{half}")
                for kc in range(KC):
                    nc.tensor.matmul(
                        ps[:],
                        cT[:, kc, mt * P:(mt + 1) * P],
                        wv_b[:, half, kc, :],
                        start=(kc == 0), stop=(kc == KC - 1))
                h0 = half * 8
                nc.vector.tensor_copy(
                    vA[:, mt, h0:h0 + 8, 0:DH],
                    ps[:].rearrange("p (h d) -> p h d", d=DH))

            # ---------------- attention ----------------
            # AV regions packed 7-per-bank into rolling psum banks.
            av_banks = {}

            def av_region(g):
                b, off = divmod(g, 7)
                if b not in av_banks:
                    av_banks[b] = psAV.tile([P, 512], F32, tag="av",
                                            name=f"avb{b}")
                return av_banks[b][:, off * 65:off * 65 + 65]

            ets = {}       # (h, kb) -> exp tile
            aqs = {}       # (hp, qb) -> normalized pair tile

            def emit_av_region(h, qb):
                # region-major: one full kb accumulation, sequential in bank
                reg = av_region(h * QB + qb)
                for kb in range(KB):
                    nc.tensor.matmul(
                        reg,
                        ets[(h, kb)][:, qb * P:(qb + 1) * P],
                        vA[:, kb, h, :],
                        start=(kb == 0), stop=(kb == KB - 1))

            def emit_norm(h, qb):
                hp, hl = divmod(h, 2)
                reg = av_region(h * QB + qb)
                rec = recp.tile([P, 1], F32, tag="rec", name=f"rec{h}_{qb}")
                if hl == 0:
                    aqs[(hp, qb)] = aqp.tile([P, P], BF16, tag="aq", name=f"aq{hp}_{qb}")
                aq = aqs[(hp, qb)]
                nc.vector.reciprocal_approx_fast(rec[:], reg[:, DH:DH + 1])
                nc.vector.tensor_scalar_mul(
                    aq[:, hl * DH:(hl + 1) * DH], reg[:, 0:DH], rec[:])
                if hl == 1:
                    nc.sync.dma_start_transpose(
                        attnT[:, hp, qb * P:(qb + 1) * P], aq[:])

            def head_block(h, extra):
                hp, hl = divmod(h, 2)
                base = hl * DH
                for kb in range(KB):
                    ps = psS.tile([P, NT], F32, tag="big", name=f"sc{h}_{kb}")
                    for n0 in (0, 512):
                        nc.tensor.matmul(
                            ps[:, n0:n0 + 512],
                            kT[base:base + DH, hp, kb * P:(kb + 1) * P],
                            qT[base:base + DH, hp, n0:n0 + 512],
                            start=True, stop=True)
                    if h > 0:
                        emit_av_region(h - 1, kb)
                    for fn in extra[kb]:
                        fn()
                    et = etp.tile([P, NT], BF16, tag="exp", name=f"et{h}_{kb}")
                    nc.scalar.activation(et[:], ps[:], EXP, scale=float(SCALE))
                    ets[(h, kb)] = et
                    if h > 0:
                        emit_norm(h - 1, kb)

            # ---------------- schedule ----------------
            kproj0()
            qproj0()
            for h in range(H):
                hp, hl = divmod(h, 2)
                extra = [[] for _ in range(KB)]
                if h == 0:
                    for mt in range(KB):
                        extra[mt].append(lambda mt=mt: vproj(mt, 0))
                if h in (1, 2):
                    for j in range(4):
                        mt = (h - 1) * 4 + j
                        extra[3 + j].append(lambda mt=mt: vproj(mt, 1))
                if hl == 0 and hp < HP - 1:
                    extra[1].append(lambda ko=hp + 1: kproj_half(ko, 0))
                    extra[2].append(lambda ko=hp + 1: kproj_half(ko, 1))
                    extra[4].append(lambda ko=hp + 1: qproj_half(ko, 0))
                elif hl == 1 and hp < HP - 1:
                    extra[1].append(lambda ko=hp + 1: qproj_half(ko, 1))
                head_block(h, extra)

            # ---------------- tail: last head's AV + out projection -----
            out3 = out_d.ap().rearrange("(t p) d -> p t d", p=P)
            for mt in range(TB):
                emit_av_region(H - 1, mt)
                emit_norm(H - 1, mt)
                ps = psS.tile([P, NT], F32, tag="big", name=f"op{mt}")
                # kc-outer so the pair-7 chunk (kc=7, fresh transpose) is
                # reached ~3us into the tile's stream; per-bank accumulation
                # streams stay sequential (n0 banks are distinct).
                for kc in range(KI):
                    for n0 in (0, 512):
                        nc.tensor.matmul(
                            ps[:, n0:n0 + 512],
                            attnT[:, kc, mt * P:(mt + 1) * P],
                            wo_b[:, kc, n0:n0 + 512],
                            start=(kc == 0), stop=(kc == KI - 1))
                ot = outp.tile([P, DQ], BF16, tag="out", name=f"ot{mt}")
                nc.vector.tensor_tensor(ot[:], ps[:], bias_b[:], ADD)
                eng = nc.sync if mt % 2 == 0 else nc.scalar
                eng.dma_start(out3[:, mt], ot[:])

            if dbg:
                nc.gpsimd.dma_start(dqT.ap(), qT[:])
                nc.gpsimd.dma_start(dkT.ap(), kT[:])
                nc.gpsimd.dma_start(dvA.ap(), vA[:])
                nc.gpsimd.dma_start(dattnT.ap(), attnT[:])

    nc.compile()
    return nc


_NC_CACHE = None


def _make_in_maps(inputs):
    import ml_dtypes
    bf = ml_dtypes.bfloat16
    x = np.asarray(inputs["x"], dtype=np.float32).astype(bf)
    context = np.asarray(inputs["context"], dtype=np.float32).astype(bf)
    wq = np.asarray(inputs["Wq"], np.float32).astype(bf)
    wk = np.asarray(inputs["Wk"], np.float32).astype(bf)
    wv = np.asarray(inputs["Wv"], np.float32).astype(bf)
    shared = {
        # [dq_chunk p, ko, kc, j]: per-(p, ko) contiguous 1536/2048B runs
        "wqpk": np.ascontiguousarray(
            wq.reshape(KQ, P, KI, P).transpose(1, 2, 0, 3)),
        "wkpk": np.ascontiguousarray(
            wk.reshape(KC, P, KI, P).transpose(1, 2, 0, 3)),
        "wvpk": np.ascontiguousarray(
            wv.reshape(KC, P, 2, 512).transpose(1, 2, 0, 3)),
        "wo": np.ascontiguousarray(np.asarray(inputs["Wo"], np.float32)
                                   .astype(bf)),
        "bo": np.ascontiguousarray(np.asarray(inputs["bo"], np.float32)
                                   .astype(bf)),
    }
    in_maps = []
    for c in range(N_CORES):
        b, s = divmod(c, 2)
        xTh = np.ascontiguousarray(x[b, s * NT:(s + 1) * NT, :].T)  # [dq, q]
        in_maps.append({
            "xpk": np.ascontiguousarray(
                xTh.reshape(KQ, P, 2, 512).transpose(1, 2, 0, 3)),
            "cT": np.ascontiguousarray(context[b].T),
            **shared,
        })
    return in_maps


def kernel(x, context, Wq, Wk, Wv, Wo, bo):
    global _NC_CACHE
    if _NC_CACHE is None:
        _NC_CACHE = build()
    nc = _NC_CACHE

    in_maps = _make_in_maps(dict(x=x, context=context, Wq=Wq, Wk=Wk, Wv=Wv,
                                 Wo=Wo, bo=bo))
    res = run_bass_kernel_spmd(nc, in_maps, core_ids=list(range(N_CORES)))
    out = np.empty((B, NQ_FULL, DQ), dtype=np.float32)
    for c in range(N_CORES):
        b, s = divmod(c, 2)
        out[b, s * NT:(s + 1) * NT, :] = res.results[c]["out"].astype(
            np.float32)
    return out


# revision 9
# speedup vs baseline: 1.1518x; 1.0170x over previous
"""CrossAttention kernel for 8 TRN2 NeuronCores.

Sharding: 8 cores = 4 batches x 2 query-halves (zero communication).
Each core computes all 16 heads for its 1024 queries.

v8 structure:
- AV computed in [q, d] orientation (lhsT = exp-scores tile, rhs = V):
  66.5k streamed columns instead of 131k for the [d, q] orientation.
  A ones-column appended to V gives the softmax denominator as column 64
  of each AV psum region -- no separate denominator matmuls.
- PSUM accumulations within one bank must be sequential (interleaving
  corrupts earlier regions), so heads are processed one at a time: head
  h's scores+exp stream in block h while head h-1's AV regions run
  region-major (kb innermost), packed 7-per-bank into 2 rolling psum
  banks one block behind.
- scores psum pool is 3-deep so the scores->exp->free chain never
  throttles the slot cadence; projections run as compact units through
  the same rotation.
- normalization is a per-partition DVE reciprocal + tensor_scalar
  multiply (q on partitions); normalized [q, 128] pair tiles go back to
  [inner, q] via DMA xbar transposes (zero PE cost), issue alternating
  between the vector and sync queues.
- input loads are tiered: critical path (cT, xT, wq0) serialized on the
  sync ring, wv on the scalar ring, bulk wk/wq chunks on the gpsimd
  ring behind a gate op that waits for cT so they cannot steal DMA
  bandwidth from the critical path.
"""

import sys

for _p in ("/opt/trn_rl_repo", "/root/.axon_site/_ro/trn_rl_repo"):
    if _p not in sys.path:
        sys.path.append(_p)

import numpy as np

import concourse.bass as bass
import concourse.tile as tile
from concourse import bacc, mybir
from concourse.bass_utils import run_bass_kernel_spmd

F32 = mybir.dt.float32
BF16 = mybir.dt.bfloat16
EXP = mybir.ActivationFunctionType.Exp
ADD = mybir.AluOpType.add

P = 128
B, NQ_FULL, DQ = 4, 2048, 1024
NK, DC = 1024, 768
H, DH = 16, 64
INNER = H * DH  # 1024
NT = 1024  # local queries per core
N_CORES = 8

KQ = DQ // P      # 8
KC = DC // P      # 6
KI = INNER // P   # 8
TB = NT // P      # 8 query tiles
KB = NK // P      # 8 kpos chunks
QB = NT // P      # 8 q-blocks for AV
HP = H // 2       # 8 head pairs
SCALE = 1.0 / np.sqrt(DH)


def build(dbg=False):
    nc = bacc.Bacc("TRN2", target_bir_lowering=False, debug=False,
                   enable_asserts=False, num_devices=N_CORES)

    cT_d = nc.dram_tensor("cT", [DC, NK], BF16, kind="ExternalInput")
    xpk_d = nc.dram_tensor("xpk", [P, 2, KQ, 512], BF16, kind="ExternalInput")
    wqpk_d = nc.dram_tensor("wqpk", [P, KI, KQ, P], BF16,
                            kind="ExternalInput")
    wkpk_d = nc.dram_tensor("wkpk", [P, KI, KC, P], BF16,
                            kind="ExternalInput")
    wvpk_d = nc.dram_tensor("wvpk", [P, 2, KC, 512], BF16,
                            kind="ExternalInput")
    wo_d = nc.dram_tensor("wo", [INNER, DQ], BF16, kind="ExternalInput")
    bo_d = nc.dram_tensor("bo", [DQ], BF16, kind="ExternalInput")
    out_d = nc.dram_tensor("out", [NT, DQ], BF16, kind="ExternalOutput")
    if dbg:
        dqT = nc.dram_tensor("dqT", [P, KI, NT], F32, kind="ExternalOutput")
        dkT = nc.dram_tensor("dkT", [P, KI, NK], F32, kind="ExternalOutput")
        dvA = nc.dram_tensor("dvA", [P, KB, H, DH + 1], F32,
                             kind="ExternalOutput")
        dattnT = nc.dram_tensor("dattnT", [P, KI, NT], F32,
                                kind="ExternalOutput")

    with tile.TileContext(nc) as tc:
        with (
            tc.tile_pool(name="persist", bufs=1) as persist,
            tc.tile_pool(name="psS", bufs=3, space="PSUM") as psS,
            tc.tile_pool(name="psAV", bufs=2, space="PSUM") as psAV,
            tc.tile_pool(name="etp", bufs=18) as etp,
            tc.tile_pool(name="aqp", bufs=20) as aqp,
            tc.tile_pool(name="recp", bufs=8) as recp,
            tc.tile_pool(name="outp", bufs=2) as outp,
        ):
            # persistent SBUF tensors
            cT = persist.tile([P, KC, NK], BF16)          # [dc, kpos]
            xT = persist.tile([P, 2, KQ, 512], BF16)      # [dq, (half,kc,q)]
            wq_b = persist.tile([P, KI, KQ, P], BF16)
            wk_b = persist.tile([P, KI, KC, P], BF16)
            wv_b = persist.tile([P, 2, KC, 512], BF16)
            wo_b = persist.tile([P, KI, DQ], BF16)
            bo_sb = persist.tile([1, DQ], BF16)
            ones_b = persist.tile([1, P], BF16)
            bias_b = persist.tile([P, DQ], BF16)          # bo bcast over parts
            gate_sb = persist.tile([1, 2, 1], BF16)       # load-tier gate dst
            qT = persist.tile([P, KI, NT], BF16)          # [inner, q]
            kT = persist.tile([P, KI, NK], BF16)          # [inner, kpos]
            vA = persist.tile([P, KB, H, DH + 1], BF16)   # [kpos,(h, d|1)]
            attnT = persist.tile([P, KI, NT], BF16)       # normalized attn^T

            # ---------------- input loads (tiered, consumer order) ------
            cT3 = cT_d.ap().rearrange("(o p) m -> p o m", p=P)
            wo4 = wo_d.ap().rearrange("(o p) m -> p o m", p=P)
            # tier 1: critical path, serialized on the sync ring
            nc.sync.dma_start(bo_sb[:], bo_d.ap()[None, :])
            nc.sync.dma_start(cT[:], cT3)
            nc.sync.dma_start(xT[:, 0], xpk_d.ap()[:, 0])
            nc.sync.dma_start(wq_b[:, 0], wqpk_d.ap()[:, 0])
            nc.sync.dma_start(xT[:, 1], xpk_d.ap()[:, 1])
            # wv on the scalar ring (idle until the first exp)
            nc.scalar.dma_start(wv_b[:, 0], wvpk_d.ap()[:, 0])
            nc.scalar.dma_start(wv_b[:, 1], wvpk_d.ap()[:, 1])
            # gpsimd ring: wk0/wk1 may race cT (tiny), the rest gated
            nc.gpsimd.memset(vA[:, :, :, DH:DH + 1], 1.0)
            nc.gpsimd.memset(ones_b[:], 1.0)
            nc.gpsimd.dma_start(wk_b[:, 0], wkpk_d.ap()[:, 0])
            nc.gpsimd.dma_start(wk_b[:, 1], wkpk_d.ap()[:, 1])
            # gate: Pool blocks here until cT fully landed, keeping the
            # bulk chunks below off the DMA engines until then
            nc.gpsimd.tensor_copy(gate_sb[:], cT[0:1, 2:4, 0:1])
            for ko in (2, 3):
                nc.gpsimd.dma_start(wk_b[:, ko], wkpk_d.ap()[:, ko])
            nc.gpsimd.dma_start(wq_b[:, 1], wqpk_d.ap()[:, 1])
            for ko in (4, 5, 6, 7):
                nc.gpsimd.dma_start(wk_b[:, ko], wkpk_d.ap()[:, ko])
                nc.gpsimd.dma_start(wq_b[:, ko - 2], wqpk_d.ap()[:, ko - 2])
            nc.gpsimd.dma_start(wq_b[:, 6], wqpk_d.ap()[:, 6])
            nc.gpsimd.dma_start(wq_b[:, 7], wqpk_d.ap()[:, 7])
            nc.gpsimd.dma_start(wo_b[:], wo4)

            # ---------------- projection units (psS rotation) -----------
            def kproj(ko):
                ps = psS.tile([P, NT], F32, tag="big", name=f"kp{ko}")
                for n0 in (0, 512):
                    for kc in range(KC):
                        nc.tensor.matmul(
                            ps[:, n0:n0 + 512],
                            wk_b[:, ko, kc, :],
                            cT[:, kc, n0:n0 + 512],
                            start=(kc == 0), stop=(kc == KC - 1))
                nc.vector.tensor_copy(kT[:, ko, :], ps[:])

            def qproj(ko):
                ps = psS.tile([P, NT], F32, tag="big", name=f"qp{ko}")
                if ko == 0:
                    # bias_b broadcast rides in this psum tile first
                    for n0 in (0, 512):
                        nc.tensor.matmul(ps[:, n0:n0 + 512], ones_b[0:1, :],
                                         bo_sb[0:1, n0:n0 + 512],
                                         start=True, stop=True)
                    nc.vector.tensor_copy(bias_b[:], ps[:])
                for hf in (0, 1):
                    n0 = hf * 512
                    for kc in range(KQ):
                        nc.tensor.matmul(
                            ps[:, n0:n0 + 512],
                            wq_b[:, ko, kc, :],
                            xT[:, hf, kc, :],
                            start=(kc == 0), stop=(kc == KQ - 1))
                nc.vector.tensor_copy(qT[:, ko, :], ps[:])

            def vproj(mt, half):
                ps = psS.tile([P, NT], F32, tag="big", name=f"vp{mt}_{half}")
                for kc in range(KC):
                    nc.tensor.matmul(
                        ps[:, 0:512],
                        cT[:, kc, mt * P:(mt + 1) * P],
                        wv_b[:, half, kc, :],
                        start=(kc == 0), stop=(kc == KC - 1))
                h0 = half * 8
                nc.vector.tensor_copy(
                    vA[:, mt, h0:h0 + 8, 0:DH],
                    ps[:, 0:512].rearrange("p (h d) -> p h d", d=DH))

            # ---------------- attention ----------------
            # AV regions packed 7-per-bank into rolling psum banks.
            av_banks = {}

            def av_region(g):
                b, off = divmod(g, 7)
                if b not in av_banks:
                    av_banks[b] = psAV.tile([P, 512], F32, tag="av",
                                            name=f"avb{b}")
                return av_banks[b][:, off * 65:off * 65 + 65]

            ets = {}       # (h, kb) -> exp tile
            aqs = {}       # (hp, qb) -> normalized pair tile
            pending_tp = []  # transposes deferred a block so sync never
                             # holds its SEQ waiting on fresh aq tiles

            def flush_tp(n=1):
                for _ in range(n):
                    if not pending_tp:
                        return
                    hp, qb = pending_tp.pop(0)
                    nc.sync.dma_start_transpose(
                        attnT[:, hp, qb * P:(qb + 1) * P],
                        aqs.pop((hp, qb))[:])

            def emit_av_region(h, qb):
                # region-major: one full kb accumulation, sequential in bank
                reg = av_region(h * QB + qb)
                for kb in range(KB):
                    nc.tensor.matmul(
                        reg,
                        ets[(h, kb)][:, qb * P:(qb + 1) * P],
                        vA[:, kb, h, :],
                        start=(kb == 0), stop=(kb == KB - 1))

            def emit_norm(h, qb):
                hp, hl = divmod(h, 2)
                reg = av_region(h * QB + qb)
                rec = recp.tile([P, 1], F32, tag="rec", name=f"rec{h}_{qb}")
                if hl == 0:
                    aqs[(hp, qb)] = aqp.tile([P, P], BF16, tag="aq",
                                             name=f"aq{hp}_{qb}")
                aq = aqs[(hp, qb)]
                nc.vector.reciprocal_approx_fast(rec[:], reg[:, DH:DH + 1])
                nc.vector.tensor_scalar_mul(
                    aq[:, hl * DH:(hl + 1) * DH], reg[:, 0:DH], rec[:])
                if hl == 1:
                    pending_tp.append((hp, qb))

            def head_block(h, extra):
                hp, hl = divmod(h, 2)
                base = hl * DH
                for kb in range(KB):
                    flush_tp(1)
                    ps = psS.tile([P, NT], F32, tag="big", name=f"sc{h}_{kb}")
                    for n0 in (0, 512):
                        nc.tensor.matmul(
                            ps[:, n0:n0 + 512],
                            kT[base:base + DH, hp, kb * P:(kb + 1) * P],
                            qT[base:base + DH, hp, n0:n0 + 512],
                            start=True, stop=True)
                    if h > 0:
                        emit_av_region(h - 1, kb)
                    for fn in extra[kb]:
                        fn()
                    et = etp.tile([P, NT], BF16, tag="exp", name=f"et{h}_{kb}")
                    nc.scalar.activation(et[:], ps[:], EXP, scale=float(SCALE))
                    ets[(h, kb)] = et
                    if h > 0:
                        emit_norm(h - 1, kb)

            # ---------------- out projection unit ------------------------
            out3 = out_d.ap().rearrange("(t p) d -> p t d", p=P)
            out_ps = {}

            def out_unit(mt, kcs, finish):
                if mt not in out_ps:
                    out_ps[mt] = psS.tile([P, NT], F32, tag="big",
                                          name=f"op{mt}")
                ps = out_ps[mt]
                # kc-outer; n0 banks see sequential accumulation streams
                for kc in kcs:
                    for n0 in (0, 512):
                        nc.tensor.matmul(
                            ps[:, n0:n0 + 512],
                            attnT[:, kc, mt * P:(mt + 1) * P],
                            wo_b[:, kc, n0:n0 + 512],
                            start=(kc == 0), stop=(finish and kc == KI - 1))
                if finish:
                    ot = outp.tile([P, DQ], BF16, tag="out", name=f"ot{mt}")
                    nc.vector.tensor_tensor(ot[:], ps[:], bias_b[:], ADD)
                    eng = nc.sync if mt % 2 == 0 else nc.scalar
                    eng.dma_start(out3[:, mt], ot[:])

            # ---------------- schedule ----------------
            kproj(0)
            qproj(0)
            for h in range(H):
                hp, hl = divmod(h, 2)
                extra = [[] for _ in range(KB)]
                if h == 0:
                    for mt in range(KB):
                        extra[mt].append(lambda mt=mt: vproj(mt, 0))
                if h in (1, 2):
                    for j in range(4):
                        mt = (h - 1) * 4 + j
                        extra[3 + j].append(lambda mt=mt: vproj(mt, 1))
                if hl == 0 and hp < HP - 1:
                    extra[1].append(lambda ko=hp + 1: kproj(ko))
                    extra[4].append(lambda ko=hp + 1: qproj(ko))
                if h == H - 1:
                    extra[3].append(lambda: out_unit(0, range(KI - 1), False))
                head_block(h, extra)

            # ---------------- tail: last head's AV + out projection -----
            for mt in range(TB):
                emit_av_region(H - 1, mt)
                emit_norm(H - 1, mt)
                flush_tp(2)
            flush_tp(len(pending_tp))
            out_unit(0, [KI - 1], True)
            for mt in range(1, TB):
                out_unit(mt, range(KI), True)

            if dbg:
                nc.gpsimd.dma_start(dqT.ap(), qT[:])
                nc.gpsimd.dma_start(dkT.ap(), kT[:])
                nc.gpsimd.dma_start(dvA.ap(), vA[:])
                nc.gpsimd.dma_start(dattnT.ap(), attnT[:])

    nc.compile()
    return nc


_NC_CACHE = None


def _make_in_maps(inputs):
    import ml_dtypes
    bf = ml_dtypes.bfloat16
    x = np.asarray(inputs["x"], dtype=np.float32).astype(bf)
    context = np.asarray(inputs["context"], dtype=np.float32).astype(bf)
    wq = np.asarray(inputs["Wq"], np.float32).astype(bf)
    wk = np.asarray(inputs["Wk"], np.float32).astype(bf)
    wv = np.asarray(inputs["Wv"], np.float32).astype(bf)
    shared = {
        # [dq_chunk p, ko, kc, j]: per-(p, ko) contiguous 1536/2048B runs
        "wqpk": np.ascontiguousarray(
            wq.reshape(KQ, P, KI, P).transpose(1, 2, 0, 3)),
        "wkpk": np.ascontiguousarray(
            wk.reshape(KC, P, KI, P).transpose(1, 2, 0, 3)),
        "wvpk": np.ascontiguousarray(
            wv.reshape(KC, P, 2, 512).transpose(1, 2, 0, 3)),
        "wo": np.ascontiguousarray(np.asarray(inputs["Wo"], np.float32)
                                   .astype(bf)),
        "bo": np.ascontiguousarray(np.asarray(inputs["bo"], np.float32)
                                   .astype(bf)),
    }
    in_maps = []
    for c in range(N_CORES):
        b, s = divmod(c, 2)
        xTh = np.ascontiguousarray(x[b, s * NT:(s + 1) * NT, :].T)  # [dq, q]
        in_maps.append({
            "xpk": np.ascontiguousarray(
                xTh.reshape(KQ, P, 2, 512).transpose(1, 2, 0, 3)),
            "cT": np.ascontiguousarray(context[b].T),
            **shared,
        })
    return in_maps


def kernel(x, context, Wq, Wk, Wv, Wo, bo):
    global _NC_CACHE
    if _NC_CACHE is None:
        _NC_CACHE = build()
    nc = _NC_CACHE

    in_maps = _make_in_maps(dict(x=x, context=context, Wq=Wq, Wk=Wk, Wv=Wv,
                                 Wo=Wo, bo=bo))
    res = run_bass_kernel_spmd(nc, in_maps, core_ids=list(range(N_CORES)))
    out = np.empty((B, NQ_FULL, DQ), dtype=np.float32)
    for c in range(N_CORES):
        b, s = divmod(c, 2)
        out[b, s * NT:(s + 1) * NT, :] = res.results[c]["out"].astype(
            np.float32)
    return out


# revision 10
# speedup vs baseline: 1.1679x; 1.0139x over previous
"""CrossAttention kernel for 8 TRN2 NeuronCores.

Sharding: 8 cores = 4 batches x 2 query-halves (zero communication).
Each core computes all 16 heads for its 1024 queries.

v8 structure:
- AV computed in [q, d] orientation (lhsT = exp-scores tile, rhs = V):
  66.5k streamed columns instead of 131k for the [d, q] orientation.
  A ones-column appended to V gives the softmax denominator as column 64
  of each AV psum region -- no separate denominator matmuls.
- PSUM accumulations within one bank must be sequential (interleaving
  corrupts earlier regions), so heads are processed one at a time: head
  h's scores+exp stream in block h while head h-1's AV regions run
  region-major (kb innermost), packed 7-per-bank into 2 rolling psum
  banks one block behind.
- scores psum pool is 3-deep so the scores->exp->free chain never
  throttles the slot cadence; projections run as compact units through
  the same rotation.
- normalization is a per-partition DVE reciprocal + tensor_scalar
  multiply (q on partitions); normalized [q, 128] pair tiles go back to
  [inner, q] via DMA xbar transposes (zero PE cost), issue alternating
  between the vector and sync queues.
- input loads are tiered: critical path (cT, xT, wq0) serialized on the
  sync ring, wv on the scalar ring, bulk wk/wq chunks on the gpsimd
  ring behind a gate op that waits for cT so they cannot steal DMA
  bandwidth from the critical path.
"""

import sys

for _p in ("/opt/trn_rl_repo", "/root/.axon_site/_ro/trn_rl_repo"):
    if _p not in sys.path:
        sys.path.append(_p)

import numpy as np

import concourse.bass as bass
import concourse.tile as tile
from concourse import bacc, mybir
from concourse.bass_utils import run_bass_kernel_spmd

F32 = mybir.dt.float32
BF16 = mybir.dt.bfloat16
EXP = mybir.ActivationFunctionType.Exp
ADD = mybir.AluOpType.add

P = 128
B, NQ_FULL, DQ = 4, 2048, 1024
NK, DC = 1024, 768
H, DH = 16, 64
INNER = H * DH  # 1024
NT = 1024  # local queries per core
N_CORES = 8

KQ = DQ // P      # 8
KC = DC // P      # 6
KI = INNER // P   # 8
TB = NT // P      # 8 query tiles
KB = NK // P      # 8 kpos chunks
QB = NT // P      # 8 q-blocks for AV
HP = H // 2       # 8 head pairs
SCALE = 1.0 / np.sqrt(DH)


def build(dbg=False):
    nc = bacc.Bacc("TRN2", target_bir_lowering=False, debug=False,
                   enable_asserts=False, num_devices=N_CORES)

    cT_d = nc.dram_tensor("cT", [DC, NK], BF16, kind="ExternalInput")
    xpk_d = nc.dram_tensor("xpk", [P, 2, KQ, 512], BF16, kind="ExternalInput")
    wqpk_d = nc.dram_tensor("wqpk", [P, KI, KQ, P], BF16,
                            kind="ExternalInput")
    wkpk_d = nc.dram_tensor("wkpk", [P, KI, KC, P], BF16,
                            kind="ExternalInput")
    wvpk_d = nc.dram_tensor("wvpk", [P, 2, KC, 512], BF16,
                            kind="ExternalInput")
    wo_d = nc.dram_tensor("wo", [INNER, DQ], BF16, kind="ExternalInput")
    bo_d = nc.dram_tensor("bo", [DQ], BF16, kind="ExternalInput")
    out_d = nc.dram_tensor("out", [NT, DQ], BF16, kind="ExternalOutput")
    if dbg:
        dqT = nc.dram_tensor("dqT", [P, KI, NT], F32, kind="ExternalOutput")
        dkT = nc.dram_tensor("dkT", [P, KI, NK], F32, kind="ExternalOutput")
        dvA = nc.dram_tensor("dvA", [P, KB, H, DH + 1], F32,
                             kind="ExternalOutput")
        dattnT = nc.dram_tensor("dattnT", [P, KI, NT], F32,
                                kind="ExternalOutput")

    with tile.TileContext(nc) as tc:
        with (
            tc.tile_pool(name="persist", bufs=1) as persist,
            tc.tile_pool(name="psS", bufs=3, space="PSUM") as psS,
            tc.tile_pool(name="psAV", bufs=2, space="PSUM") as psAV,
            tc.tile_pool(name="etp", bufs=18) as etp,
            tc.tile_pool(name="aqp", bufs=20) as aqp,
            tc.tile_pool(name="recp", bufs=8) as recp,
            tc.tile_pool(name="outp", bufs=2) as outp,
        ):
            # persistent SBUF tensors
            cT = persist.tile([P, KC, NK], BF16)          # [dc, kpos]
            xT = persist.tile([P, 2, KQ, 512], BF16)      # [dq, (half,kc,q)]
            wq_b = persist.tile([P, KI, KQ, P], BF16)
            wk_b = persist.tile([P, KI, KC, P], BF16)
            wv_b = persist.tile([P, 2, KC, 512], BF16)
            wo_b = persist.tile([P, KI, DQ], BF16)
            bo_sb = persist.tile([1, DQ], BF16)
            ones_b = persist.tile([1, P], BF16)
            bias_b = persist.tile([P, DQ], BF16)          # bo bcast over parts
            qT = persist.tile([P, KI, NT], BF16)          # [inner, q]
            kT = persist.tile([P, KI, NK], BF16)          # [inner, kpos]
            vA = persist.tile([P, KB, H, DH + 1], BF16)   # [kpos,(h, d|1)]
            attnT = persist.tile([P, KI, NT], BF16)       # normalized attn^T

            # ---------------- input loads (tiered, consumer order) ------
            # The scheduler keeps emission order among ready DMAs per queue,
            # so the critical path (cT -> xT/wq0) leads all three DMA-capable
            # rings; bulk wk/wq chunks trail on the gpsimd ring.
            cT3 = cT_d.ap().rearrange("(o p) m -> p o m", p=P)
            wo4 = wo_d.ap().rearrange("(o p) m -> p o m", p=P)
            nc.sync.dma_start(bo_sb[:], bo_d.ap()[None, :])
            nc.sync.dma_start(cT[:, 0:2], cT3[:, 0:2])
            nc.sync.dma_start(xT[:, 0, 0:4], xpk_d.ap()[:, 0, 0:4])
            nc.sync.dma_start(xT[:, 1, 0:4], xpk_d.ap()[:, 1, 0:4])
            nc.scalar.dma_start(cT[:, 2:4], cT3[:, 2:4])
            nc.scalar.dma_start(xT[:, 0, 4:8], xpk_d.ap()[:, 0, 4:8])
            nc.scalar.dma_start(wq_b[:, 0], wqpk_d.ap()[:, 0])
            nc.scalar.dma_start(xT[:, 1, 4:8], xpk_d.ap()[:, 1, 4:8])
            nc.scalar.dma_start(wv_b[:, 0], wvpk_d.ap()[:, 0])
            nc.scalar.dma_start(wv_b[:, 1], wvpk_d.ap()[:, 1])
            nc.gpsimd.memset(vA[:, :, :, DH:DH + 1], 1.0)
            nc.gpsimd.memset(ones_b[:], 1.0)
            nc.gpsimd.dma_start(cT[:, 4:6], cT3[:, 4:6])
            nc.gpsimd.dma_start(wk_b[:, 0], wkpk_d.ap()[:, 0])
            nc.gpsimd.dma_start(wk_b[:, 1], wkpk_d.ap()[:, 1])
            nc.gpsimd.dma_start(wk_b[:, 2], wkpk_d.ap()[:, 2])
            nc.gpsimd.dma_start(wq_b[:, 1], wqpk_d.ap()[:, 1])
            for ko in (3, 4, 5, 6, 7):
                nc.gpsimd.dma_start(wk_b[:, ko], wkpk_d.ap()[:, ko])
                nc.gpsimd.dma_start(wq_b[:, ko - 1], wqpk_d.ap()[:, ko - 1])
            nc.gpsimd.dma_start(wq_b[:, 7], wqpk_d.ap()[:, 7])
            nc.gpsimd.dma_start(wo_b[:], wo4)

            # ---------------- projection units (psS rotation) -----------
            def kproj(ko):
                ps = psS.tile([P, NT], F32, tag="big", name=f"kp{ko}")
                for n0 in (0, 512):
                    for kc in range(KC):
                        nc.tensor.matmul(
                            ps[:, n0:n0 + 512],
                            wk_b[:, ko, kc, :],
                            cT[:, kc, n0:n0 + 512],
                            start=(kc == 0), stop=(kc == KC - 1))
                nc.vector.tensor_copy(kT[:, ko, :], ps[:])

            def qproj(ko):
                ps = psS.tile([P, NT], F32, tag="big", name=f"qp{ko}")
                if ko == 0:
                    # bias_b broadcast rides in this psum tile first
                    for n0 in (0, 512):
                        nc.tensor.matmul(ps[:, n0:n0 + 512], ones_b[0:1, :],
                                         bo_sb[0:1, n0:n0 + 512],
                                         start=True, stop=True)
                    nc.vector.tensor_copy(bias_b[:], ps[:])
                for hf in (0, 1):
                    n0 = hf * 512
                    for kc in range(KQ):
                        nc.tensor.matmul(
                            ps[:, n0:n0 + 512],
                            wq_b[:, ko, kc, :],
                            xT[:, hf, kc, :],
                            start=(kc == 0), stop=(kc == KQ - 1))
                    nc.vector.tensor_copy(qT[:, ko, n0:n0 + 512],
                                          ps[:, n0:n0 + 512])

            def vproj(mt, half):
                ps = psS.tile([P, NT], F32, tag="big", name=f"vp{mt}_{half}")
                for kc in range(KC):
                    nc.tensor.matmul(
                        ps[:, 0:512],
                        cT[:, kc, mt * P:(mt + 1) * P],
                        wv_b[:, half, kc, :],
                        start=(kc == 0), stop=(kc == KC - 1))
                h0 = half * 8
                nc.vector.tensor_copy(
                    vA[:, mt, h0:h0 + 8, 0:DH],
                    ps[:, 0:512].rearrange("p (h d) -> p h d", d=DH))

            # ---------------- attention ----------------
            # AV regions packed 7-per-bank into rolling psum banks.
            av_banks = {}

            def av_region(g):
                b, off = divmod(g, 7)
                if b not in av_banks:
                    av_banks[b] = psAV.tile([P, 512], F32, tag="av",
                                            name=f"avb{b}")
                return av_banks[b][:, off * 65:off * 65 + 65]

            ets = {}       # (h, kb) -> exp tile
            aqs = {}       # (hp, qb) -> normalized pair tile
            pending_tp = []  # transposes deferred a block so sync never
                             # holds its SEQ waiting on fresh aq tiles

            def flush_tp(n=1):
                for _ in range(n):
                    if not pending_tp:
                        return
                    hp, qb = pending_tp.pop(0)
                    nc.sync.dma_start_transpose(
                        attnT[:, hp, qb * P:(qb + 1) * P],
                        aqs.pop((hp, qb))[:])

            def emit_av_region(h, qb):
                # region-major: one full kb accumulation, sequential in bank
                reg = av_region(h * QB + qb)
                for kb in range(KB):
                    nc.tensor.matmul(
                        reg,
                        ets[(h, kb)][:, qb * P:(qb + 1) * P],
                        vA[:, kb, h, :],
                        start=(kb == 0), stop=(kb == KB - 1))

            def emit_norm(h, qb):
                hp, hl = divmod(h, 2)
                reg = av_region(h * QB + qb)
                rec = recp.tile([P, 1], F32, tag="rec", name=f"rec{h}_{qb}")
                if hl == 0:
                    aqs[(hp, qb)] = aqp.tile([P, P], BF16, tag="aq",
                                             name=f"aq{hp}_{qb}")
                aq = aqs[(hp, qb)]
                nc.vector.reciprocal_approx_fast(rec[:], reg[:, DH:DH + 1])
                nc.vector.tensor_scalar_mul(
                    aq[:, hl * DH:(hl + 1) * DH], reg[:, 0:DH], rec[:])
                if hl == 1:
                    pending_tp.append((hp, qb))

            def head_block(h, extra):
                hp, hl = divmod(h, 2)
                base = hl * DH
                for kb in range(KB):
                    flush_tp(1)
                    ps = psS.tile([P, NT], F32, tag="big", name=f"sc{h}_{kb}")
                    for n0 in (0, 512):
                        nc.tensor.matmul(
                            ps[:, n0:n0 + 512],
                            kT[base:base + DH, hp, kb * P:(kb + 1) * P],
                            qT[base:base + DH, hp, n0:n0 + 512],
                            start=True, stop=True)
                    # cluster AV regions (4 at a time) to cut PE stream
                    # switches; region-major order within banks preserved
                    if h > 0 and kb in (1, 5):
                        for qb in range(kb - 1, kb + 3):
                            emit_av_region(h - 1, qb)
                    for fn in extra[kb]:
                        fn()
                    et = etp.tile([P, NT], BF16, tag="exp", name=f"et{h}_{kb}")
                    nc.scalar.activation(et[:], ps[:], EXP, scale=float(SCALE))
                    ets[(h, kb)] = et
                    if h > 0 and kb in (1, 5):
                        for qb in range(kb - 1, kb + 3):
                            emit_norm(h - 1, qb)

            # ---------------- out projection unit ------------------------
            out3 = out_d.ap().rearrange("(t p) d -> p t d", p=P)
            out_ps = {}

            def out_unit(mt, kcs, finish):
                if mt not in out_ps:
                    out_ps[mt] = psS.tile([P, NT], F32, tag="big",
                                          name=f"op{mt}")
                ps = out_ps[mt]
                # kc-outer; n0 banks see sequential accumulation streams
                for kc in kcs:
                    for n0 in (0, 512):
                        nc.tensor.matmul(
                            ps[:, n0:n0 + 512],
                            attnT[:, kc, mt * P:(mt + 1) * P],
                            wo_b[:, kc, n0:n0 + 512],
                            start=(kc == 0), stop=(finish and kc == KI - 1))
                if finish:
                    ot = outp.tile([P, DQ], BF16, tag="out", name=f"ot{mt}")
                    nc.vector.tensor_tensor(ot[:], ps[:], bias_b[:], ADD)
                    eng = nc.sync if mt % 2 == 0 else nc.scalar
                    eng.dma_start(out3[:, mt], ot[:])

            # ---------------- schedule ----------------
            kproj(0)
            qproj(0)
            for h in range(H):
                hp, hl = divmod(h, 2)
                extra = [[] for _ in range(KB)]
                if h == 0:
                    for mt in range(KB):
                        extra[mt].append(lambda mt=mt: vproj(mt, 0))
                if h in (1, 2):
                    for j in range(4):
                        mt = (h - 1) * 4 + j
                        extra[3 + j].append(lambda mt=mt: vproj(mt, 1))
                if hl == 0 and hp < HP - 1:
                    extra[1].append(lambda ko=hp + 1: kproj(ko))
                    extra[4].append(lambda ko=hp + 1: qproj(ko))
                if h == H - 1:
                    extra[3].append(lambda: out_unit(0, range(KI - 1), False))
                head_block(h, extra)

            # ---------------- tail: last head's AV + out projection -----
            for mt in range(TB):
                emit_av_region(H - 1, mt)
                emit_norm(H - 1, mt)
                flush_tp(2)
            flush_tp(len(pending_tp))
            out_unit(0, [KI - 1], True)
            for mt in range(1, TB):
                out_unit(mt, range(KI), True)

            if dbg:
                nc.gpsimd.dma_start(dqT.ap(), qT[:])
                nc.gpsimd.dma_start(dkT.ap(), kT[:])
                nc.gpsimd.dma_start(dvA.ap(), vA[:])
                nc.gpsimd.dma_start(dattnT.ap(), attnT[:])

    nc.compile()
    return nc


_NC_CACHE = None


def _make_in_maps(inputs):
    import ml_dtypes
    bf = ml_dtypes.bfloat16
    x = np.asarray(inputs["x"], dtype=np.float32).astype(bf)
    context = np.asarray(inputs["context"], dtype=np.float32).astype(bf)
    wq = np.asarray(inputs["Wq"], np.float32).astype(bf)
    wk = np.asarray(inputs["Wk"], np.float32).astype(bf)
    wv = np.asarray(inputs["Wv"], np.float32).astype(bf)
    shared = {
        # [dq_chunk p, ko, kc, j]: per-(p, ko) contiguous 1536/2048B runs
        "wqpk": np.ascontiguousarray(
            wq.reshape(KQ, P, KI, P).transpose(1, 2, 0, 3)),
        "wkpk": np.ascontiguousarray(
            wk.reshape(KC, P, KI, P).transpose(1, 2, 0, 3)),
        "wvpk": np.ascontiguousarray(
            wv.reshape(KC, P, 2, 512).transpose(1, 2, 0, 3)),
        "wo": np.ascontiguousarray(np.asarray(inputs["Wo"], np.float32)
                                   .astype(bf)),
        "bo": np.ascontiguousarray(np.asarray(inputs["bo"], np.float32)
                                   .astype(bf)),
    }
    in_maps = []
    for c in range(N_CORES):
        b, s = divmod(c, 2)
        xTh = np.ascontiguousarray(x[b, s * NT:(s + 1) * NT, :].T)  # [dq, q]
        in_maps.append({
            "xpk": np.ascontiguousarray(
                xTh.reshape(KQ, P, 2, 512).transpose(1, 2, 0, 3)),
            "cT": np.ascontiguousarray(context[b].T),
            **shared,
        })
    return in_maps


def kernel(x, context, Wq, Wk, Wv, Wo, bo):
    global _NC_CACHE
    if _NC_CACHE is None:
        _NC_CACHE = build()
    nc = _NC_CACHE

    in_maps = _make_in_maps(dict(x=x, context=context, Wq=Wq, Wk=Wk, Wv=Wv,
                                 Wo=Wo, bo=bo))
    res = run_bass_kernel_spmd(nc, in_maps, core_ids=list(range(N_CORES)))
    out = np.empty((B, NQ_FULL, DQ), dtype=np.float32)
    for c in range(N_CORES):
        b, s = divmod(c, 2)
        out[b, s * NT:(s + 1) * NT, :] = res.results[c]["out"].astype(
            np.float32)
    return out


# revision 11
# speedup vs baseline: 1.1840x; 1.0138x over previous
"""CrossAttention kernel for 8 TRN2 NeuronCores.

Sharding: 8 cores = 4 batches x 2 query-halves (zero communication).
Each core computes all 16 heads for its 1024 queries.

v8 structure:
- AV computed in [q, d] orientation (lhsT = exp-scores tile, rhs = V):
  66.5k streamed columns instead of 131k for the [d, q] orientation.
  A ones-column appended to V gives the softmax denominator as column 64
  of each AV psum region -- no separate denominator matmuls.
- PSUM accumulations within one bank must be sequential (interleaving
  corrupts earlier regions), so heads are processed one at a time: head
  h's scores+exp stream in block h while head h-1's AV regions run
  region-major (kb innermost), packed 7-per-bank into 2 rolling psum
  banks one block behind.
- scores psum pool is 3-deep so the scores->exp->free chain never
  throttles the slot cadence; projections run as compact units through
  the same rotation.
- normalization is a per-partition DVE reciprocal + tensor_scalar
  multiply (q on partitions); normalized [q, 128] pair tiles go back to
  [inner, q] via DMA xbar transposes (zero PE cost), issue alternating
  between the vector and sync queues.
- input loads are tiered: critical path (cT, xT, wq0) serialized on the
  sync ring, wv on the scalar ring, bulk wk/wq chunks on the gpsimd
  ring behind a gate op that waits for cT so they cannot steal DMA
  bandwidth from the critical path.
"""

import sys

for _p in ("/opt/trn_rl_repo", "/root/.axon_site/_ro/trn_rl_repo"):
    if _p not in sys.path:
        sys.path.append(_p)

import numpy as np

import concourse.bass as bass
import concourse.tile as tile
from concourse import bacc, mybir
from concourse.bass_utils import run_bass_kernel_spmd

F32 = mybir.dt.float32
BF16 = mybir.dt.bfloat16
EXP = mybir.ActivationFunctionType.Exp
ADD = mybir.AluOpType.add

P = 128
B, NQ_FULL, DQ = 4, 2048, 1024
NK, DC = 1024, 768
H, DH = 16, 64
INNER = H * DH  # 1024
NT = 1024  # local queries per core
N_CORES = 8

KQ = DQ // P      # 8
KC = DC // P      # 6
KI = INNER // P   # 8
TB = NT // P      # 8 query tiles
KB = NK // P      # 8 kpos chunks
QB = NT // P      # 8 q-blocks for AV
HP = H // 2       # 8 head pairs
SCALE = 1.0 / np.sqrt(DH)


def build(dbg=False):
    nc = bacc.Bacc("TRN2", target_bir_lowering=False, debug=False,
                   enable_asserts=False, num_devices=N_CORES)

    cT_d = nc.dram_tensor("cT", [DC, NK], BF16, kind="ExternalInput")
    xpk_d = nc.dram_tensor("xpk", [P, 2, KQ, 512], BF16, kind="ExternalInput")
    wqpk_d = nc.dram_tensor("wqpk", [P, KI, KQ, P], BF16,
                            kind="ExternalInput")
    wkpk_d = nc.dram_tensor("wkpk", [P, KI, KC, P], BF16,
                            kind="ExternalInput")
    wvpk_d = nc.dram_tensor("wvpk", [P, 2, KC, 512], BF16,
                            kind="ExternalInput")
    wo_d = nc.dram_tensor("wo", [INNER, DQ], BF16, kind="ExternalInput")
    bo_d = nc.dram_tensor("bo", [DQ], BF16, kind="ExternalInput")
    out_d = nc.dram_tensor("out", [NT, DQ], BF16, kind="ExternalOutput")
    if dbg:
        dqT = nc.dram_tensor("dqT", [P, KI, NT], F32, kind="ExternalOutput")
        dkT = nc.dram_tensor("dkT", [P, KI, NK], F32, kind="ExternalOutput")
        dvA = nc.dram_tensor("dvA", [P, KB, H, DH + 1], F32,
                             kind="ExternalOutput")
        dattnT = nc.dram_tensor("dattnT", [P, KI, NT], F32,
                                kind="ExternalOutput")

    with tile.TileContext(nc) as tc:
        with (
            tc.tile_pool(name="persist", bufs=1) as persist,
            tc.tile_pool(name="psS", bufs=3, space="PSUM") as psS,
            tc.tile_pool(name="psAV", bufs=2, space="PSUM") as psAV,
            tc.tile_pool(name="etp", bufs=18) as etp,
            tc.tile_pool(name="aqp", bufs=20) as aqp,
            tc.tile_pool(name="recp", bufs=8) as recp,
            tc.tile_pool(name="outp", bufs=2) as outp,
        ):
            # persistent SBUF tensors
            cT = persist.tile([P, KC, NK], BF16)          # [dc, kpos]
            xT = persist.tile([P, 2, KQ, 512], BF16)      # [dq, (half,kc,q)]
            wq_b = persist.tile([P, KI, KQ, P], BF16)
            wk_b = persist.tile([P, KI, KC, P], BF16)
            wv_b = persist.tile([P, 2, KC, 512], BF16)
            wo_b = persist.tile([P, KI, DQ], BF16)
            bo_sb = persist.tile([1, DQ], BF16)
            ones_b = persist.tile([1, P], BF16)
            bias_b = persist.tile([P, DQ], BF16)          # bo bcast over parts
            qT = persist.tile([P, KI, NT], BF16)          # [inner, q]
            kT = persist.tile([P, KI, NK], BF16)          # [inner, kpos]
            vA = persist.tile([P, KB, H, DH + 1], BF16)   # [kpos,(h, d|1)]
            attnT = persist.tile([P, KI, NT], BF16)       # normalized attn^T

            # ---------------- input loads (tiered, consumer order) ------
            # The scheduler keeps emission order among ready DMAs per queue,
            # so the critical path (cT -> xT/wq0) leads all three DMA-capable
            # rings; bulk wk/wq chunks trail on the gpsimd ring.
            cT3 = cT_d.ap().rearrange("(o p) m -> p o m", p=P)
            wo4 = wo_d.ap().rearrange("(o p) m -> p o m", p=P)
            nc.sync.dma_start(bo_sb[:], bo_d.ap()[None, :])
            nc.sync.dma_start(cT[:, 0:2], cT3[:, 0:2])
            nc.sync.dma_start(xT[:, 0, 0:4], xpk_d.ap()[:, 0, 0:4])
            nc.sync.dma_start(xT[:, 1, 0:4], xpk_d.ap()[:, 1, 0:4])
            nc.scalar.dma_start(cT[:, 2:4], cT3[:, 2:4])
            nc.scalar.dma_start(wq_b[:, 0], wqpk_d.ap()[:, 0])
            nc.scalar.dma_start(xT[:, 0, 4:8], xpk_d.ap()[:, 0, 4:8])
            nc.scalar.dma_start(xT[:, 1, 4:8], xpk_d.ap()[:, 1, 4:8])
            nc.scalar.dma_start(wv_b[:, 0], wvpk_d.ap()[:, 0])
            nc.scalar.dma_start(wv_b[:, 1], wvpk_d.ap()[:, 1])
            nc.gpsimd.memset(vA[:, :, :, DH:DH + 1], 1.0)
            nc.gpsimd.memset(ones_b[:], 1.0)
            nc.gpsimd.dma_start(wk_b[:, 0], wkpk_d.ap()[:, 0])
            nc.gpsimd.dma_start(cT[:, 4:6], cT3[:, 4:6])
            nc.gpsimd.dma_start(wk_b[:, 1], wkpk_d.ap()[:, 1])
            nc.gpsimd.dma_start(wk_b[:, 2], wkpk_d.ap()[:, 2])
            nc.gpsimd.dma_start(wq_b[:, 1], wqpk_d.ap()[:, 1])
            for ko in (3, 4, 5, 6, 7):
                nc.gpsimd.dma_start(wk_b[:, ko], wkpk_d.ap()[:, ko])
                nc.gpsimd.dma_start(wq_b[:, ko - 1], wqpk_d.ap()[:, ko - 1])
            nc.gpsimd.dma_start(wq_b[:, 7], wqpk_d.ap()[:, 7])
            nc.gpsimd.dma_start(wo_b[:], wo4)

            # ---------------- projection units (psS rotation) -----------
            def kproj(ko):
                ps = psS.tile([P, NT], F32, tag="big", name=f"kp{ko}")
                # ko=0 runs during the input loads: accumulate in the order
                # the cT thirds land (scalar, sync, gpsimd rings)
                kcs = (2, 3, 0, 1, 4, 5) if ko == 0 else tuple(range(KC))
                for n0 in (0, 512):
                    for i, kc in enumerate(kcs):
                        nc.tensor.matmul(
                            ps[:, n0:n0 + 512],
                            wk_b[:, ko, kc, :],
                            cT[:, kc, n0:n0 + 512],
                            start=(i == 0), stop=(i == KC - 1))
                nc.vector.tensor_copy(kT[:, ko, :], ps[:])

            def qproj(ko):
                ps = psS.tile([P, NT], F32, tag="big", name=f"qp{ko}")
                if ko == 0:
                    # bias_b broadcast rides in this psum tile first
                    for n0 in (0, 512):
                        nc.tensor.matmul(ps[:, n0:n0 + 512], ones_b[0:1, :],
                                         bo_sb[0:1, n0:n0 + 512],
                                         start=True, stop=True)
                    nc.vector.tensor_copy(bias_b[:], ps[:])
                for hf in (0, 1):
                    n0 = hf * 512
                    for kc in range(KQ):
                        nc.tensor.matmul(
                            ps[:, n0:n0 + 512],
                            wq_b[:, ko, kc, :],
                            xT[:, hf, kc, :],
                            start=(kc == 0), stop=(kc == KQ - 1))
                    nc.vector.tensor_copy(qT[:, ko, n0:n0 + 512],
                                          ps[:, n0:n0 + 512])

            def vproj(mt, half):
                ps = psS.tile([P, NT], F32, tag="big", name=f"vp{mt}_{half}")
                for kc in range(KC):
                    nc.tensor.matmul(
                        ps[:, 0:512],
                        cT[:, kc, mt * P:(mt + 1) * P],
                        wv_b[:, half, kc, :],
                        start=(kc == 0), stop=(kc == KC - 1))
                h0 = half * 8
                nc.vector.tensor_copy(
                    vA[:, mt, h0:h0 + 8, 0:DH],
                    ps[:, 0:512].rearrange("p (h d) -> p h d", d=DH))

            # ---------------- attention ----------------
            # AV regions packed 7-per-bank into rolling psum banks.
            av_banks = {}

            def av_region(g):
                b, off = divmod(g, 7)
                if b not in av_banks:
                    av_banks[b] = psAV.tile([P, 512], F32, tag="av",
                                            name=f"avb{b}")
                return av_banks[b][:, off * 65:off * 65 + 65]

            ets = {}       # (h, kb) -> exp tile
            aqs = {}       # (hp, qb) -> normalized pair tile
            pending_tp = []  # transposes deferred a block so sync never
                             # holds its SEQ waiting on fresh aq tiles

            def flush_tp(n=1):
                for _ in range(n):
                    if not pending_tp:
                        return
                    hp, qb = pending_tp.pop(0)
                    nc.sync.dma_start_transpose(
                        attnT[:, hp, qb * P:(qb + 1) * P],
                        aqs.pop((hp, qb))[:])

            def emit_av_region(h, qb):
                # region-major: one full kb accumulation, sequential in bank
                reg = av_region(h * QB + qb)
                for kb in range(KB):
                    nc.tensor.matmul(
                        reg,
                        ets[(h, kb)][:, qb * P:(qb + 1) * P],
                        vA[:, kb, h, :],
                        start=(kb == 0), stop=(kb == KB - 1))

            def emit_norm(h, qb):
                hp, hl = divmod(h, 2)
                reg = av_region(h * QB + qb)
                rec = recp.tile([P, 1], F32, tag="rec", name=f"rec{h}_{qb}")
                if hl == 0:
                    aqs[(hp, qb)] = aqp.tile([P, P], BF16, tag="aq",
                                             name=f"aq{hp}_{qb}")
                aq = aqs[(hp, qb)]
                nc.vector.reciprocal_approx_fast(rec[:], reg[:, DH:DH + 1])
                nc.vector.tensor_scalar_mul(
                    aq[:, hl * DH:(hl + 1) * DH], reg[:, 0:DH], rec[:])
                if hl == 1:
                    pending_tp.append((hp, qb))

            def head_block(h, extra):
                hp, hl = divmod(h, 2)
                base = hl * DH
                for kb in range(KB):
                    flush_tp(1)
                    ps = psS.tile([P, NT], F32, tag="big", name=f"sc{h}_{kb}")
                    for n0 in (0, 512):
                        nc.tensor.matmul(
                            ps[:, n0:n0 + 512],
                            kT[base:base + DH, hp, kb * P:(kb + 1) * P],
                            qT[base:base + DH, hp, n0:n0 + 512],
                            start=True, stop=True)
                    # cluster all AV regions in one stream to cut PE
                    # switches; region-major order within banks preserved
                    if h > 0 and kb == 1:
                        for qb in range(QB):
                            emit_av_region(h - 1, qb)
                    for fn in extra[kb]:
                        fn()
                    et = etp.tile([P, NT], BF16, tag="exp", name=f"et{h}_{kb}")
                    nc.scalar.activation(et[:], ps[:], EXP, scale=float(SCALE))
                    ets[(h, kb)] = et
                    if h > 0 and kb == 1:
                        for qb in range(QB):
                            emit_norm(h - 1, qb)

            # ---------------- out projection unit ------------------------
            out3 = out_d.ap().rearrange("(t p) d -> p t d", p=P)
            out_ps = {}

            def out_unit(mt, kcs, finish):
                if mt not in out_ps:
                    out_ps[mt] = psS.tile([P, NT], F32, tag="big",
                                          name=f"op{mt}")
                ps = out_ps[mt]
                # kc-outer; n0 banks see sequential accumulation streams
                for kc in kcs:
                    for n0 in (0, 512):
                        nc.tensor.matmul(
                            ps[:, n0:n0 + 512],
                            attnT[:, kc, mt * P:(mt + 1) * P],
                            wo_b[:, kc, n0:n0 + 512],
                            start=(kc == 0), stop=(finish and kc == KI - 1))
                if finish:
                    ot = outp.tile([P, DQ], BF16, tag="out", name=f"ot{mt}")
                    nc.vector.tensor_tensor(ot[:], ps[:], bias_b[:], ADD)
                    eng = nc.sync if mt % 2 == 0 else nc.scalar
                    eng.dma_start(out3[:, mt], ot[:])

            # ---------------- schedule ----------------
            kproj(0)
            qproj(0)
            for h in range(H):
                hp, hl = divmod(h, 2)
                extra = [[] for _ in range(KB)]
                if h == 0:
                    for mt in range(KB):
                        extra[mt].append(lambda mt=mt: vproj(mt, 0))
                if h in (1, 2):
                    for j in range(4):
                        mt = (h - 1) * 4 + j
                        extra[3 + j].append(lambda mt=mt: vproj(mt, 1))
                if hl == 0 and hp < HP - 1:
                    extra[1].append(lambda ko=hp + 1: kproj(ko))
                    extra[4].append(lambda ko=hp + 1: qproj(ko))
                if h == H - 1:
                    extra[3].append(lambda: out_unit(0, range(KI - 1), False))
                head_block(h, extra)

            # ---------------- tail: last head's AV + out projection -----
            for mt in range(TB):
                emit_av_region(H - 1, mt)
                emit_norm(H - 1, mt)
                flush_tp(2)
            flush_tp(len(pending_tp))
            out_unit(0, [KI - 1], True)
            for mt in range(1, TB):
                out_unit(mt, range(KI), True)

            if dbg:
                nc.gpsimd.dma_start(dqT.ap(), qT[:])
                nc.gpsimd.dma_start(dkT.ap(), kT[:])
                nc.gpsimd.dma_start(dvA.ap(), vA[:])
                nc.gpsimd.dma_start(dattnT.ap(), attnT[:])

    nc.compile()
    return nc


_NC_CACHE = None


def _make_in_maps(inputs):
    import ml_dtypes
    bf = ml_dtypes.bfloat16
    x = np.asarray(inputs["x"], dtype=np.float32).astype(bf)
    context = np.asarray(inputs["context"], dtype=np.float32).astype(bf)
    wq = np.asarray(inputs["Wq"], np.float32).astype(bf)
    wk = np.asarray(inputs["Wk"], np.float32).astype(bf)
    wv = np.asarray(inputs["Wv"], np.float32).astype(bf)
    shared = {
        # [dq_chunk p, ko, kc, j]: per-(p, ko) contiguous 1536/2048B runs
        "wqpk": np.ascontiguousarray(
            wq.reshape(KQ, P, KI, P).transpose(1, 2, 0, 3)),
        "wkpk": np.ascontiguousarray(
            wk.reshape(KC, P, KI, P).transpose(1, 2, 0, 3)),
        "wvpk": np.ascontiguousarray(
            wv.reshape(KC, P, 2, 512).transpose(1, 2, 0, 3)),
        "wo": np.ascontiguousarray(np.asarray(inputs["Wo"], np.float32)
                                   .astype(bf)),
        "bo": np.ascontiguousarray(np.asarray(inputs["bo"], np.float32)
                                   .astype(bf)),
    }
    in_maps = []
    for c in range(N_CORES):
        b, s = divmod(c, 2)
        xTh = np.ascontiguousarray(x[b, s * NT:(s + 1) * NT, :].T)  # [dq, q]
        in_maps.append({
            "xpk": np.ascontiguousarray(
                xTh.reshape(KQ, P, 2, 512).transpose(1, 2, 0, 3)),
            "cT": np.ascontiguousarray(context[b].T),
            **shared,
        })
    return in_maps


def kernel(x, context, Wq, Wk, Wv, Wo, bo):
    global _NC_CACHE
    if _NC_CACHE is None:
        _NC_CACHE = build()
    nc = _NC_CACHE

    in_maps = _make_in_maps(dict(x=x, context=context, Wq=Wq, Wk=Wk, Wv=Wv,
                                 Wo=Wo, bo=bo))
    res = run_bass_kernel_spmd(nc, in_maps, core_ids=list(range(N_CORES)))
    out = np.empty((B, NQ_FULL, DQ), dtype=np.float32)
    for c in range(N_CORES):
        b, s = divmod(c, 2)
        out[b, s * NT:(s + 1) * NT, :] = res.results[c]["out"].astype(
            np.float32)
    return out


# revision 13
# speedup vs baseline: 1.2739x; 1.0759x over previous
"""CrossAttention kernel for 8 TRN2 NeuronCores.

Sharding: 8 cores = 4 batches x 2 query-halves (zero communication).
Each core computes all 16 heads for its 1024 queries.

v8 structure:
- AV computed in [q, d] orientation (lhsT = exp-scores tile, rhs = V):
  66.5k streamed columns instead of 131k for the [d, q] orientation.
  A ones-column appended to V gives the softmax denominator as column 64
  of each AV psum region -- no separate denominator matmuls.
- PSUM accumulations within one bank must be sequential (interleaving
  corrupts earlier regions), so heads are processed one at a time: head
  h's scores+exp stream in block h while head h-1's AV regions run
  region-major (kb innermost), packed 7-per-bank into 2 rolling psum
  banks one block behind.
- scores psum pool is 3-deep so the scores->exp->free chain never
  throttles the slot cadence; projections run as compact units through
  the same rotation.
- normalization is a per-partition DVE reciprocal + tensor_scalar
  multiply (q on partitions); normalized [q, 128] pair tiles go back to
  [inner, q] via DMA xbar transposes (zero PE cost), issue alternating
  between the vector and sync queues.
- input loads are tiered: critical path (cT, xT, wq0) serialized on the
  sync ring, wv on the scalar ring, bulk wk/wq chunks on the gpsimd
  ring behind a gate op that waits for cT so they cannot steal DMA
  bandwidth from the critical path.
"""

import sys

for _p in ("/opt/trn_rl_repo", "/root/.axon_site/_ro/trn_rl_repo"):
    if _p not in sys.path:
        sys.path.append(_p)

import numpy as np

import concourse.bass as bass
import concourse.tile as tile
from concourse import bacc, mybir
from concourse.bass_utils import run_bass_kernel_spmd

F32 = mybir.dt.float32
BF16 = mybir.dt.bfloat16
EXP = mybir.ActivationFunctionType.Exp
ADD = mybir.AluOpType.add

P = 128
B, NQ_FULL, DQ = 4, 2048, 1024
NK, DC = 1024, 768
H, DH = 16, 64
INNER = H * DH  # 1024
NT = 1024  # local queries per core
N_CORES = 8

KQ = DQ // P      # 8
KC = DC // P      # 6
KI = INNER // P   # 8
TB = NT // P      # 8 query tiles
KB = NK // P      # 8 kpos chunks
QB = NT // P      # 8 q-blocks for AV
HP = H // 2       # 8 head pairs
SCALE = 1.0 / np.sqrt(DH)


def build(dbg=False):
    nc = bacc.Bacc("TRN2", target_bir_lowering=False, debug=False,
                   enable_asserts=False, num_devices=N_CORES)

    cT_d = nc.dram_tensor("cT", [DC, NK], BF16, kind="ExternalInput")
    xpk_d = nc.dram_tensor("xpk", [P, 2, KQ, 512], BF16, kind="ExternalInput")
    wqpk_d = nc.dram_tensor("wqpk", [P, KI, KQ, P], BF16,
                            kind="ExternalInput")
    wkpk_d = nc.dram_tensor("wkpk", [P, KI, KC, P], BF16,
                            kind="ExternalInput")
    wvpk_d = nc.dram_tensor("wvpk", [P, 2, KC, 512], BF16,
                            kind="ExternalInput")
    wo_d = nc.dram_tensor("wo", [INNER, DQ], BF16, kind="ExternalInput")
    bo_d = nc.dram_tensor("bo", [DQ], BF16, kind="ExternalInput")
    out_d = nc.dram_tensor("out", [NT, DQ], BF16, kind="ExternalOutput")
    if dbg:
        dqT = nc.dram_tensor("dqT", [P, KI, NT], F32, kind="ExternalOutput")
        dkT = nc.dram_tensor("dkT", [P, KI, NK], F32, kind="ExternalOutput")
        dvA = nc.dram_tensor("dvA", [P, KB, H, DH + 1], F32,
                             kind="ExternalOutput")
        dattnT = nc.dram_tensor("dattnT", [P, KI, NT], F32,
                                kind="ExternalOutput")

    with tile.TileContext(nc) as tc:
        with (
            tc.tile_pool(name="persist", bufs=1) as persist,
            tc.tile_pool(name="psS", bufs=3, space="PSUM") as psS,
            tc.tile_pool(name="psAV", bufs=2, space="PSUM") as psAV,
            tc.tile_pool(name="etp", bufs=18) as etp,
            tc.tile_pool(name="aqp", bufs=20) as aqp,
            tc.tile_pool(name="recp", bufs=8) as recp,
            tc.tile_pool(name="outp", bufs=2) as outp,
        ):
            # persistent SBUF tensors
            cT = persist.tile([P, KC, NK], BF16)          # [dc, kpos]
            xT = persist.tile([P, 2, KQ, 512], BF16)      # [dq, (half,kc,q)]
            wq_b = persist.tile([P, KI, KQ, P], BF16)
            wk_b = persist.tile([P, KI, KC, P], BF16)
            wv_b = persist.tile([P, 2, KC, 512], BF16)
            wo_b = persist.tile([P, KI, DQ], BF16)
            bo_sb = persist.tile([1, DQ], BF16)
            ones_b = persist.tile([1, P], BF16)
            bias_b = persist.tile([P, DQ], BF16)          # bo bcast over parts
            qT = persist.tile([P, KI, NT], BF16)          # [inner, q]
            kT = persist.tile([P, KI, NK], BF16)          # [inner, kpos]
            vA = persist.tile([P, KB, H, DH + 1], BF16)   # [kpos,(h, d|1)]
            attnT = persist.tile([P, KI, NT], BF16)       # normalized attn^T

            # ---------------- input loads (tiered, consumer order) ------
            # The scheduler keeps emission order among ready DMAs per queue,
            # so the critical path (cT -> xT/wq0) leads all three DMA-capable
            # rings; bulk wk/wq chunks trail on the gpsimd ring.
            cT3 = cT_d.ap().rearrange("(o p) m -> p o m", p=P)
            wo4 = wo_d.ap().rearrange("(o p) m -> p o m", p=P)
            nc.sync.dma_start(bo_sb[:], bo_d.ap()[None, :])
            nc.sync.dma_start(cT[:, 0:2], cT3[:, 0:2])
            nc.sync.dma_start(xT[:, 0, 0:4], xpk_d.ap()[:, 0, 0:4])
            nc.sync.dma_start(xT[:, 1, 0:4], xpk_d.ap()[:, 1, 0:4])
            nc.scalar.dma_start(cT[:, 2:4], cT3[:, 2:4])
            nc.scalar.dma_start(wq_b[:, 0], wqpk_d.ap()[:, 0])
            nc.scalar.dma_start(xT[:, 0, 4:8], xpk_d.ap()[:, 0, 4:8])
            nc.scalar.dma_start(xT[:, 1, 4:8], xpk_d.ap()[:, 1, 4:8])
            nc.scalar.dma_start(wv_b[:, 0], wvpk_d.ap()[:, 0])
            nc.scalar.dma_start(wv_b[:, 1], wvpk_d.ap()[:, 1])
            nc.gpsimd.memset(vA[:, :, :, DH:DH + 1], 1.0)
            nc.gpsimd.memset(ones_b[:], 1.0)
            nc.gpsimd.dma_start(wk_b[:, 0], wkpk_d.ap()[:, 0])
            nc.gpsimd.dma_start(cT[:, 4:6], cT3[:, 4:6])
            nc.gpsimd.dma_start(wk_b[:, 1], wkpk_d.ap()[:, 1])
            nc.gpsimd.dma_start(wk_b[:, 2], wkpk_d.ap()[:, 2])
            nc.gpsimd.dma_start(wq_b[:, 1], wqpk_d.ap()[:, 1])
            for ko in (3, 4, 5, 6, 7):
                nc.gpsimd.dma_start(wk_b[:, ko], wkpk_d.ap()[:, ko])
                nc.gpsimd.dma_start(wq_b[:, ko - 1], wqpk_d.ap()[:, ko - 1])
            nc.gpsimd.dma_start(wq_b[:, 7], wqpk_d.ap()[:, 7])
            nc.gpsimd.dma_start(wo_b[:], wo4)

            # ---------------- projection units (psS rotation) -----------
            def kproj(ko):
                ps = psS.tile([P, NT], F32, tag="big", name=f"kp{ko}")
                # ko=0 runs during the input loads: accumulate in the order
                # the cT thirds land (scalar, sync, gpsimd rings)
                kcs = (2, 3, 0, 1, 4, 5) if ko == 0 else tuple(range(KC))
                for n0 in (0, 512):
                    for i, kc in enumerate(kcs):
                        nc.tensor.matmul(
                            ps[:, n0:n0 + 512],
                            wk_b[:, ko, kc, :],
                            cT[:, kc, n0:n0 + 512],
                            start=(i == 0), stop=(i == KC - 1))
                nc.vector.tensor_copy(kT[:, ko, :], ps[:])

            def qproj(ko):
                ps = psS.tile([P, NT], F32, tag="big", name=f"qp{ko}")
                if ko == 0:
                    # bias_b broadcast rides in this psum tile first
                    for n0 in (0, 512):
                        nc.tensor.matmul(ps[:, n0:n0 + 512], ones_b[0:1, :],
                                         bo_sb[0:1, n0:n0 + 512],
                                         start=True, stop=True)
                    nc.vector.tensor_copy(bias_b[:], ps[:])
                for hf in (0, 1):
                    n0 = hf * 512
                    for kc in range(KQ):
                        nc.tensor.matmul(
                            ps[:, n0:n0 + 512],
                            wq_b[:, ko, kc, :],
                            xT[:, hf, kc, :],
                            start=(kc == 0), stop=(kc == KQ - 1))
                    nc.vector.tensor_copy(qT[:, ko, n0:n0 + 512],
                                          ps[:, n0:n0 + 512])

            def vproj(mt, half):
                ps = psS.tile([P, NT], F32, tag="big", name=f"vp{mt}_{half}")
                for kc in range(KC):
                    nc.tensor.matmul(
                        ps[:, 0:512],
                        cT[:, kc, mt * P:(mt + 1) * P],
                        wv_b[:, half, kc, :],
                        start=(kc == 0), stop=(kc == KC - 1))
                h0 = half * 8
                nc.vector.tensor_copy(
                    vA[:, mt, h0:h0 + 8, 0:DH],
                    ps[:, 0:512].rearrange("p (h d) -> p h d", d=DH))

            # ---------------- attention ----------------
            # AV regions packed 7-per-bank into rolling psum banks.
            av_banks = {}

            def av_region(g):
                b, off = divmod(g, 7)
                if b not in av_banks:
                    av_banks[b] = psAV.tile([P, 512], F32, tag="av",
                                            name=f"avb{b}")
                return av_banks[b][:, off * 65:off * 65 + 65]

            ets = {}       # (h, kb) -> exp tile
            aqs = {}       # (hp, qb) -> normalized pair tile
            pending_tp = []  # transposes deferred a block so sync never
                             # holds its SEQ waiting on fresh aq tiles

            def flush_tp(n=1):
                for _ in range(n):
                    if not pending_tp:
                        return
                    hp, qb = pending_tp.pop(0)
                    nc.sync.dma_start_transpose(
                        attnT[:, hp, qb * P:(qb + 1) * P],
                        aqs.pop((hp, qb))[:])

            def emit_av_region(h, qb):
                # region-major: one full kb accumulation, sequential in bank
                reg = av_region(h * QB + qb)
                for kb in range(KB):
                    nc.tensor.matmul(
                        reg,
                        ets[(h, kb)][:, qb * P:(qb + 1) * P],
                        vA[:, kb, h, :],
                        start=(kb == 0), stop=(kb == KB - 1))

            def emit_norm(h, qb):
                hp, hl = divmod(h, 2)
                reg = av_region(h * QB + qb)
                rec = recp.tile([P, 1], F32, tag="rec", name=f"rec{h}_{qb}")
                if hl == 0:
                    aqs[(hp, qb)] = aqp.tile([P, P], BF16, tag="aq",
                                             name=f"aq{hp}_{qb}")
                aq = aqs[(hp, qb)]
                nc.vector.reciprocal_approx_fast(rec[:], reg[:, DH:DH + 1])
                nc.vector.tensor_scalar_mul(
                    aq[:, hl * DH:(hl + 1) * DH], reg[:, 0:DH], rec[:])
                if hl == 1:
                    pending_tp.append((hp, qb))

            def head_block(h, extra):
                hp, hl = divmod(h, 2)
                base = hl * DH
                for kb in range(KB):
                    flush_tp(1)
                    ps = psS.tile([P, NT], F32, tag="big", name=f"sc{h}_{kb}")
                    for n0 in (0, 512):
                        nc.tensor.matmul(
                            ps[:, n0:n0 + 512],
                            kT[base:base + DH, hp, kb * P:(kb + 1) * P],
                            qT[base:base + DH, hp, n0:n0 + 512],
                            start=True, stop=True)
                    # cluster all AV regions in one stream to cut PE
                    # switches; region-major order within banks preserved
                    if h > 0 and kb == 1:
                        for qb in range(QB):
                            emit_av_region(h - 1, qb)
                    for fn in extra[kb]:
                        fn()
                    et = etp.tile([P, NT], BF16, tag="exp", name=f"et{h}_{kb}")
                    nc.scalar.activation(et[:], ps[:], EXP, scale=float(SCALE))
                    ets[(h, kb)] = et
                    if h > 0 and kb == 1:
                        for qb in range(QB):
                            emit_norm(h - 1, qb)

            # ---------------- out projection unit ------------------------
            out3 = out_d.ap().rearrange("(t p) d -> p t d", p=P)
            out_ps = {}

            def out_unit(mt, kcs, finish):
                if mt not in out_ps:
                    out_ps[mt] = psS.tile([P, NT], F32, tag="big",
                                          name=f"op{mt}")
                ps = out_ps[mt]
                # kc-outer; n0 banks see sequential accumulation streams
                for kc in kcs:
                    for n0 in (0, 512):
                        nc.tensor.matmul(
                            ps[:, n0:n0 + 512],
                            attnT[:, kc, mt * P:(mt + 1) * P],
                            wo_b[:, kc, n0:n0 + 512],
                            start=(kc == 0), stop=(finish and kc == KI - 1))
                if finish:
                    ot = outp.tile([P, DQ], BF16, tag="out", name=f"ot{mt}")
                    nc.vector.tensor_tensor(ot[:], ps[:], bias_b[:], ADD)
                    eng = nc.sync if mt % 2 == 0 else nc.scalar
                    eng.dma_start(out3[:, mt], ot[:])

            # ---------------- schedule ----------------
            kproj(0)
            qproj(0)
            for h in range(H):
                hp, hl = divmod(h, 2)
                extra = [[] for _ in range(KB)]
                if h == 0:
                    for mt in range(KB):
                        extra[mt].append(lambda mt=mt: vproj(mt, 0))
                    extra[4].append(lambda: kproj(1))
                if h in (1, 2):
                    for j in range(4):
                        mt = (h - 1) * 4 + j
                        extra[(5 + j) % KB].append(lambda mt=mt: vproj(mt, 1))
                # balanced projection placement: kproj on even blocks,
                # qproj on odd blocks, away from the slot-1 AV cluster
                if hl == 0 and 0 < hp < HP - 1:
                    extra[4].append(lambda ko=hp + 1: kproj(ko))
                if hl == 1 and hp < HP - 1:
                    extra[4].append(lambda ko=hp + 1: qproj(ko))
                if h == H - 1:
                    extra[3].append(lambda: out_unit(0, range(KI - 1), False))
                head_block(h, extra)

            # ---------------- tail: last head's AV + out projection -----
            for mt in range(TB):
                emit_av_region(H - 1, mt)
                emit_norm(H - 1, mt)
                flush_tp(2)
            flush_tp(len(pending_tp))
            out_unit(0, [KI - 1], True)
            for mt in range(1, TB):
                out_unit(mt, range(KI), True)

            if dbg:
                nc.gpsimd.dma_start(dqT.ap(), qT[:])
                nc.gpsimd.dma_start(dkT.ap(), kT[:])
                nc.gpsimd.dma_start(dvA.ap(), vA[:])
                nc.gpsimd.dma_start(dattnT.ap(), attnT[:])

    nc.compile()
    return nc


_NC_CACHE = None


def _make_in_maps(inputs):
    import ml_dtypes
    bf = ml_dtypes.bfloat16
    x = np.asarray(inputs["x"], dtype=np.float32).astype(bf)
    context = np.asarray(inputs["context"], dtype=np.float32).astype(bf)
    wq = np.asarray(inputs["Wq"], np.float32).astype(bf)
    wk = np.asarray(inputs["Wk"], np.float32).astype(bf)
    wv = np.asarray(inputs["Wv"], np.float32).astype(bf)
    shared = {
        # [dq_chunk p, ko, kc, j]: per-(p, ko) contiguous 1536/2048B runs
        "wqpk": np.ascontiguousarray(
            wq.reshape(KQ, P, KI, P).transpose(1, 2, 0, 3)),
        "wkpk": np.ascontiguousarray(
            wk.reshape(KC, P, KI, P).transpose(1, 2, 0, 3)),
        "wvpk": np.ascontiguousarray(
            wv.reshape(KC, P, 2, 512).transpose(1, 2, 0, 3)),
        "wo": np.ascontiguousarray(np.asarray(inputs["Wo"], np.float32)
                                   .astype(bf)),
        "bo": np.ascontiguousarray(np.asarray(inputs["bo"], np.float32)
                                   .astype(bf)),
    }
    in_maps = []
    for c in range(N_CORES):
        b, s = divmod(c, 2)
        xTh = np.ascontiguousarray(x[b, s * NT:(s + 1) * NT, :].T)  # [dq, q]
        in_maps.append({
            "xpk": np.ascontiguousarray(
                xTh.reshape(KQ, P, 2, 512).transpose(1, 2, 0, 3)),
            "cT": np.ascontiguousarray(context[b].T),
            **shared,
        })
    return in_maps


def kernel(x, context, Wq, Wk, Wv, Wo, bo):
    global _NC_CACHE
    if _NC_CACHE is None:
        _NC_CACHE = build()
    nc = _NC_CACHE

    in_maps = _make_in_maps(dict(x=x, context=context, Wq=Wq, Wk=Wk, Wv=Wv,
                                 Wo=Wo, bo=bo))
    res = run_bass_kernel_spmd(nc, in_maps, core_ids=list(range(N_CORES)))
    out = np.empty((B, NQ_FULL, DQ), dtype=np.float32)
    for c in range(N_CORES):
        b, s = divmod(c, 2)
        out[b, s * NT:(s + 1) * NT, :] = res.results[c]["out"].astype(
            np.float32)
    return out


# revision 15
# speedup vs baseline: 1.2871x; 1.0104x over previous
"""CrossAttention kernel for 8 TRN2 NeuronCores.

Sharding: 8 cores = 4 batches x 2 query-halves (zero communication).
Each core computes all 16 heads for its 1024 queries.

v8 structure:
- AV computed in [q, d] orientation (lhsT = exp-scores tile, rhs = V):
  66.5k streamed columns instead of 131k for the [d, q] orientation.
  A ones-column appended to V gives the softmax denominator as column 64
  of each AV psum region -- no separate denominator matmuls.
- PSUM accumulations within one bank must be sequential (interleaving
  corrupts earlier regions), so heads are processed one at a time: head
  h's scores+exp stream in block h while head h-1's AV regions run
  region-major (kb innermost), packed 7-per-bank into 2 rolling psum
  banks one block behind.
- scores psum pool is 3-deep so the scores->exp->free chain never
  throttles the slot cadence; projections run as compact units through
  the same rotation.
- normalization is a per-partition DVE reciprocal + tensor_scalar
  multiply (q on partitions); normalized [q, 128] pair tiles go back to
  [inner, q] via DMA xbar transposes (zero PE cost), issue alternating
  between the vector and sync queues.
- input loads are tiered: critical path (cT, xT, wq0) serialized on the
  sync ring, wv on the scalar ring, bulk wk/wq chunks on the gpsimd
  ring behind a gate op that waits for cT so they cannot steal DMA
  bandwidth from the critical path.
"""

import sys

for _p in ("/opt/trn_rl_repo", "/root/.axon_site/_ro/trn_rl_repo"):
    if _p not in sys.path:
        sys.path.append(_p)

import numpy as np

import concourse.bass as bass
import concourse.tile as tile
from concourse import bacc, mybir
from concourse.bass_utils import run_bass_kernel_spmd

F32 = mybir.dt.float32
BF16 = mybir.dt.bfloat16
EXP = mybir.ActivationFunctionType.Exp
ADD = mybir.AluOpType.add

P = 128
B, NQ_FULL, DQ = 4, 2048, 1024
NK, DC = 1024, 768
H, DH = 16, 64
INNER = H * DH  # 1024
NT = 1024  # local queries per core
N_CORES = 8

KQ = DQ // P      # 8
KC = DC // P      # 6
KI = INNER // P   # 8
TB = NT // P      # 8 query tiles
KB = NK // P      # 8 kpos chunks
QB = NT // P      # 8 q-blocks for AV
HP = H // 2       # 8 head pairs
SCALE = 1.0 / np.sqrt(DH)


def build(dbg=False):
    nc = bacc.Bacc("TRN2", target_bir_lowering=False, debug=False,
                   enable_asserts=False, num_devices=N_CORES)

    cT_d = nc.dram_tensor("cT", [DC, NK], BF16, kind="ExternalInput")
    xpk_d = nc.dram_tensor("xpk", [P, 2, KQ, 512], BF16, kind="ExternalInput")
    wqpk_d = nc.dram_tensor("wqpk", [P, KI, KQ, P], BF16,
                            kind="ExternalInput")
    wkpk_d = nc.dram_tensor("wkpk", [P, KI, KC, P], BF16,
                            kind="ExternalInput")
    wvpk_d = nc.dram_tensor("wvpk", [P, 2, KC, 512], BF16,
                            kind="ExternalInput")
    wo_d = nc.dram_tensor("wo", [INNER, DQ], BF16, kind="ExternalInput")
    bo_d = nc.dram_tensor("bo", [DQ], BF16, kind="ExternalInput")
    out_d = nc.dram_tensor("out", [NT, DQ], BF16, kind="ExternalOutput")
    if dbg:
        dqT = nc.dram_tensor("dqT", [P, KI, NT], F32, kind="ExternalOutput")
        dkT = nc.dram_tensor("dkT", [P, KI, NK], F32, kind="ExternalOutput")
        dvA = nc.dram_tensor("dvA", [P, KB, H, DH + 1], F32,
                             kind="ExternalOutput")
        dattnT = nc.dram_tensor("dattnT", [P, KI, NT], F32,
                                kind="ExternalOutput")

    with tile.TileContext(nc) as tc:
        with (
            tc.tile_pool(name="persist", bufs=1) as persist,
            tc.tile_pool(name="psS", bufs=3, space="PSUM") as psS,
            tc.tile_pool(name="psAV", bufs=2, space="PSUM") as psAV,
            tc.tile_pool(name="etp", bufs=18) as etp,
            tc.tile_pool(name="aqp", bufs=20) as aqp,
            tc.tile_pool(name="recp", bufs=8) as recp,
            tc.tile_pool(name="outp", bufs=2) as outp,
        ):
            # persistent SBUF tensors
            cT = persist.tile([P, KC, NK], BF16)          # [dc, kpos]
            xT = persist.tile([P, 2, KQ, 512], BF16)      # [dq, (half,kc,q)]
            wq_b = persist.tile([P, KI, KQ, P], BF16)
            wk_b = persist.tile([P, KI, KC, P], BF16)
            wv_b = persist.tile([P, 2, KC, 512], BF16)
            wo_b = persist.tile([P, KI, DQ], BF16)
            bo_sb = persist.tile([1, DQ], BF16)
            ones_b = persist.tile([1, P], BF16)
            bias_b = persist.tile([P, DQ], BF16)          # bo bcast over parts
            qT = persist.tile([P, KI, NT], BF16)          # [inner, q]
            kT = persist.tile([P, KI, NK], BF16)          # [inner, kpos]
            vA = persist.tile([P, KB, H, DH + 1], BF16)   # [kpos,(h, d|1)]
            attnT = persist.tile([P, KI, NT], BF16)       # normalized attn^T

            # ---------------- input loads (tiered, consumer order) ------
            # The scheduler keeps emission order among ready DMAs per queue,
            # so the critical path (cT -> xT/wq0) leads all three DMA-capable
            # rings; bulk wk/wq chunks trail on the gpsimd ring.
            cT3 = cT_d.ap().rearrange("(o p) m -> p o m", p=P)
            wo4 = wo_d.ap().rearrange("(o p) m -> p o m", p=P)
            nc.sync.dma_start(bo_sb[:], bo_d.ap()[None, :])
            nc.sync.dma_start(cT[:, 0:2], cT3[:, 0:2])
            nc.sync.dma_start(xT[:, 0, 0:4], xpk_d.ap()[:, 0, 0:4])
            nc.sync.dma_start(xT[:, 1, 0:4], xpk_d.ap()[:, 1, 0:4])
            nc.scalar.dma_start(cT[:, 2:4], cT3[:, 2:4])
            nc.scalar.dma_start(wq_b[:, 0], wqpk_d.ap()[:, 0])
            nc.scalar.dma_start(xT[:, 0, 4:8], xpk_d.ap()[:, 0, 4:8])
            nc.scalar.dma_start(xT[:, 1, 4:8], xpk_d.ap()[:, 1, 4:8])
            nc.scalar.dma_start(wv_b[:, 0], wvpk_d.ap()[:, 0])
            nc.scalar.dma_start(wv_b[:, 1], wvpk_d.ap()[:, 1])
            nc.gpsimd.memset(vA[:, :, :, DH:DH + 1], 1.0)
            nc.gpsimd.memset(ones_b[:], 1.0)
            nc.gpsimd.dma_start(wk_b[:, 0], wkpk_d.ap()[:, 0])
            nc.gpsimd.dma_start(cT[:, 4:6], cT3[:, 4:6])
            nc.gpsimd.dma_start(wk_b[:, 1], wkpk_d.ap()[:, 1])
            nc.gpsimd.dma_start(wk_b[:, 2], wkpk_d.ap()[:, 2])
            nc.gpsimd.dma_start(wq_b[:, 1], wqpk_d.ap()[:, 1])
            for ko in (3, 4, 5, 6, 7):
                nc.gpsimd.dma_start(wk_b[:, ko], wkpk_d.ap()[:, ko])
                nc.gpsimd.dma_start(wq_b[:, ko - 1], wqpk_d.ap()[:, ko - 1])
            nc.gpsimd.dma_start(wq_b[:, 7], wqpk_d.ap()[:, 7])
            nc.gpsimd.dma_start(wo_b[:], wo4)

            # ---------------- projection units (psS rotation) -----------
            def kproj(ko):
                ps = psS.tile([P, NT], F32, tag="big", name=f"kp{ko}")
                # ko=0 runs during the input loads: accumulate in the order
                # the cT thirds land (scalar, sync, gpsimd rings)
                kcs = (2, 3, 0, 1, 4, 5) if ko == 0 else tuple(range(KC))
                for n0 in (0, 512):
                    for i, kc in enumerate(kcs):
                        nc.tensor.matmul(
                            ps[:, n0:n0 + 512],
                            wk_b[:, ko, kc, :],
                            cT[:, kc, n0:n0 + 512],
                            start=(i == 0), stop=(i == KC - 1))
                nc.vector.tensor_copy(kT[:, ko, :], ps[:])

            def qproj(ko):
                ps = psS.tile([P, NT], F32, tag="big", name=f"qp{ko}")
                if ko == 0:
                    # bias_b broadcast rides in this psum tile first
                    for n0 in (0, 512):
                        nc.tensor.matmul(ps[:, n0:n0 + 512], ones_b[0:1, :],
                                         bo_sb[0:1, n0:n0 + 512],
                                         start=True, stop=True)
                    nc.vector.tensor_copy(bias_b[:], ps[:])
                for hf in (0, 1):
                    n0 = hf * 512
                    for kc in range(KQ):
                        nc.tensor.matmul(
                            ps[:, n0:n0 + 512],
                            wq_b[:, ko, kc, :],
                            xT[:, hf, kc, :],
                            start=(kc == 0), stop=(kc == KQ - 1))
                    nc.vector.tensor_copy(qT[:, ko, n0:n0 + 512],
                                          ps[:, n0:n0 + 512])

            def vproj(mt, half):
                ps = psS.tile([P, NT], F32, tag="big", name=f"vp{mt}_{half}")
                for kc in range(KC):
                    nc.tensor.matmul(
                        ps[:, 0:512],
                        cT[:, kc, mt * P:(mt + 1) * P],
                        wv_b[:, half, kc, :],
                        start=(kc == 0), stop=(kc == KC - 1))
                h0 = half * 8
                nc.vector.tensor_copy(
                    vA[:, mt, h0:h0 + 8, 0:DH],
                    ps[:, 0:512].rearrange("p (h d) -> p h d", d=DH))

            # ---------------- attention ----------------
            # AV regions packed 7-per-bank into rolling psum banks.
            av_banks = {}

            def av_region(g):
                b, off = divmod(g, 7)
                if b not in av_banks:
                    av_banks[b] = psAV.tile([P, 512], F32, tag="av",
                                            name=f"avb{b}")
                return av_banks[b][:, off * 65:off * 65 + 65]

            ets = {}       # (h, kb) -> exp tile
            aqs = {}       # (hp, qb) -> normalized pair tile
            pending_tp = []  # transposes deferred a block so sync never
                             # holds its SEQ waiting on fresh aq tiles

            def flush_tp(n=1):
                for _ in range(n):
                    if not pending_tp:
                        return
                    hp, qb = pending_tp.pop(0)
                    nc.sync.dma_start_transpose(
                        attnT[:, hp, qb * P:(qb + 1) * P],
                        aqs.pop((hp, qb))[:])

            def emit_av_region(h, qb):
                # region-major: one full kb accumulation, sequential in bank
                reg = av_region(h * QB + qb)
                for kb in range(KB):
                    nc.tensor.matmul(
                        reg,
                        ets[(h, kb)][:, qb * P:(qb + 1) * P],
                        vA[:, kb, h, :],
                        start=(kb == 0), stop=(kb == KB - 1))

            def emit_norm(h, qb):
                hp, hl = divmod(h, 2)
                reg = av_region(h * QB + qb)
                rec = recp.tile([P, 1], F32, tag="rec", name=f"rec{h}_{qb}")
                if hl == 0:
                    aqs[(hp, qb)] = aqp.tile([P, P], BF16, tag="aq",
                                             name=f"aq{hp}_{qb}")
                aq = aqs[(hp, qb)]
                nc.vector.reciprocal_approx_fast(rec[:], reg[:, DH:DH + 1])
                nc.vector.tensor_scalar_mul(
                    aq[:, hl * DH:(hl + 1) * DH], reg[:, 0:DH], rec[:])
                if hl == 1:
                    pending_tp.append((hp, qb))

            def head_block(h, extra):
                hp, hl = divmod(h, 2)
                base = hl * DH
                for kb in range(KB):
                    flush_tp(1)
                    ps = psS.tile([P, NT], F32, tag="big", name=f"sc{h}_{kb}")
                    for n0 in (0, 512):
                        nc.tensor.matmul(
                            ps[:, n0:n0 + 512],
                            kT[base:base + DH, hp, kb * P:(kb + 1) * P],
                            qT[base:base + DH, hp, n0:n0 + 512],
                            start=True, stop=True)
                    # cluster all AV regions in one stream to cut PE
                    # switches; region-major order within banks preserved
                    if h > 0 and kb == 1:
                        for qb in range(QB):
                            emit_av_region(h - 1, qb)
                    for fn in extra[kb]:
                        fn()
                    et = etp.tile([P, NT], BF16, tag="exp", name=f"et{h}_{kb}")
                    nc.scalar.activation(et[:], ps[:], EXP, scale=float(SCALE))
                    ets[(h, kb)] = et
                    if h > 0 and kb == 1:
                        for qb in range(QB):
                            emit_norm(h - 1, qb)

            # ---------------- out projection unit ------------------------
            out3 = out_d.ap().rearrange("(t p) d -> p t d", p=P)
            out_ps = {}

            def out_unit(mt, kcs, finish):
                if mt not in out_ps:
                    out_ps[mt] = psS.tile([P, NT], F32, tag="big",
                                          name=f"op{mt}")
                ps = out_ps[mt]
                # kc-outer; n0 banks see sequential accumulation streams
                for kc in kcs:
                    for n0 in (0, 512):
                        nc.tensor.matmul(
                            ps[:, n0:n0 + 512],
                            attnT[:, kc, mt * P:(mt + 1) * P],
                            wo_b[:, kc, n0:n0 + 512],
                            start=(kc == 0), stop=(finish and kc == KI - 1))
                if finish:
                    ot = outp.tile([P, DQ], BF16, tag="out", name=f"ot{mt}")
                    nc.vector.tensor_tensor(ot[:], ps[:], bias_b[:], ADD)
                    eng = nc.sync if mt % 2 == 0 else nc.scalar
                    eng.dma_start(out3[:, mt], ot[:])

            # ---------------- schedule ----------------
            # interleave the first k/q projections so the PE fills the
            # wait for the last cT third / xT halves with useful work
            kp0 = psS.tile([P, NT], F32, tag="big", name="kp0")
            qp0 = psS.tile([P, NT], F32, tag="big", name="qp0")
            for n0 in (0, 512):
                for i, kc in enumerate((2, 3, 0, 1)):
                    nc.tensor.matmul(kp0[:, n0:n0 + 512], wk_b[:, 0, kc, :],
                                     cT[:, kc, n0:n0 + 512],
                                     start=(i == 0), stop=False)
            bp0 = psS.tile([P, NT], F32, tag="big", name="bp0")
            for n0 in (0, 512):
                nc.tensor.matmul(bp0[:, n0:n0 + 512], ones_b[0:1, :],
                                 bo_sb[0:1, n0:n0 + 512],
                                 start=True, stop=True)
            nc.vector.tensor_copy(bias_b[:], bp0[:])
            for kc in range(KQ):
                nc.tensor.matmul(qp0[:, 0:512], wq_b[:, 0, kc, :],
                                 xT[:, 0, kc, :],
                                 start=(kc == 0), stop=(kc == KQ - 1))
            nc.vector.tensor_copy(qT[:, 0, 0:512], qp0[:, 0:512])
            for n0 in (0, 512):
                for i, kc in enumerate((4, 5)):
                    nc.tensor.matmul(kp0[:, n0:n0 + 512], wk_b[:, 0, kc, :],
                                     cT[:, kc, n0:n0 + 512],
                                     start=False, stop=(i == 1))
            nc.vector.tensor_copy(kT[:, 0, :], kp0[:])
            for kc in range(KQ):
                nc.tensor.matmul(qp0[:, 512:1024], wq_b[:, 0, kc, :],
                                 xT[:, 1, kc, :],
                                 start=(kc == 0), stop=(kc == KQ - 1))
            nc.vector.tensor_copy(qT[:, 0, 512:1024], qp0[:, 512:1024])
            for h in range(H):
                hp, hl = divmod(h, 2)
                extra = [[] for _ in range(KB)]
                if h == 0:
                    for mt in range(KB):
                        extra[mt].append(lambda mt=mt: vproj(mt, 0))
                    extra[4].append(lambda: kproj(1))
                if 1 <= h <= 8:
                    mt = h - 1
                    extra[6 if h < 8 else 4].append(
                        lambda mt=mt: vproj(mt, 1))
                # balanced projection placement: kproj on even blocks,
                # qproj on odd blocks, away from the slot-1 AV cluster
                if hl == 0 and 0 < hp < HP - 1:
                    extra[4].append(lambda ko=hp + 1: kproj(ko))
                if hl == 1 and hp < HP - 1:
                    extra[4].append(lambda ko=hp + 1: qproj(ko))
                if h == H - 1:
                    extra[3].append(lambda: out_unit(0, range(KI - 1), False))
                head_block(h, extra)

            # ---------------- tail: last head's AV + out projection -----
            for mt in range(TB):
                emit_av_region(H - 1, mt)
                emit_norm(H - 1, mt)
                flush_tp(2)
            flush_tp(len(pending_tp))
            out_unit(0, [KI - 1], True)
            for mt in range(1, TB):
                out_unit(mt, range(KI), True)

            if dbg:
                nc.gpsimd.dma_start(dqT.ap(), qT[:])
                nc.gpsimd.dma_start(dkT.ap(), kT[:])
                nc.gpsimd.dma_start(dvA.ap(), vA[:])
                nc.gpsimd.dma_start(dattnT.ap(), attnT[:])

    nc.compile()
    return nc


_NC_CACHE = None


def _make_in_maps(inputs):
    import ml_dtypes
    bf = ml_dtypes.bfloat16
    x = np.asarray(inputs["x"], dtype=np.float32).astype(bf)
    context = np.asarray(inputs["context"], dtype=np.float32).astype(bf)
    wq = np.asarray(inputs["Wq"], np.float32).astype(bf)
    wk = np.asarray(inputs["Wk"], np.float32).astype(bf)
    wv = np.asarray(inputs["Wv"], np.float32).astype(bf)
    shared = {
        # [dq_chunk p, ko, kc, j]: per-(p, ko) contiguous 1536/2048B runs
        "wqpk": np.ascontiguousarray(
            wq.reshape(KQ, P, KI, P).transpose(1, 2, 0, 3)),
        "wkpk": np.ascontiguousarray(
            wk.reshape(KC, P, KI, P).transpose(1, 2, 0, 3)),
        "wvpk": np.ascontiguousarray(
            wv.reshape(KC, P, 2, 512).transpose(1, 2, 0, 3)),
        "wo": np.ascontiguousarray(np.asarray(inputs["Wo"], np.float32)
                                   .astype(bf)),
        "bo": np.ascontiguousarray(np.asarray(inputs["bo"], np.float32)
                                   .astype(bf)),
    }
    in_maps = []
    for c in range(N_CORES):
        b, s = divmod(c, 2)
        xTh = np.ascontiguousarray(x[b, s * NT:(s + 1) * NT, :].T)  # [dq, q]
        in_maps.append({
            "xpk": np.ascontiguousarray(
                xTh.reshape(KQ, P, 2, 512).transpose(1, 2, 0, 3)),
            "cT": np.ascontiguousarray(context[b].T),
            **shared,
        })
    return in_maps


def kernel(x, context, Wq, Wk, Wv, Wo, bo):
    global _NC_CACHE
    if _NC_CACHE is None:
        _NC_CACHE = build()
    nc = _NC_CACHE

    in_maps = _make_in_maps(dict(x=x, context=context, Wq=Wq, Wk=Wk, Wv=Wv,
                                 Wo=Wo, bo=bo))
    res = run_bass_kernel_spmd(nc, in_maps, core_ids=list(range(N_CORES)))
    out = np.empty((B, NQ_FULL, DQ), dtype=np.float32)
    for c in range(N_CORES):
        b, s = divmod(c, 2)
        out[b, s * NT:(s + 1) * NT, :] = res.results[c]["out"].astype(
            np.float32)
    return out


# revision 16
# speedup vs baseline: 1.2951x; 1.0063x over previous
"""CrossAttention kernel for 8 TRN2 NeuronCores.

Sharding: 8 cores = 4 batches x 2 query-halves (zero communication).
Each core computes all 16 heads for its 1024 queries.

v8 structure:
- AV computed in [q, d] orientation (lhsT = exp-scores tile, rhs = V):
  66.5k streamed columns instead of 131k for the [d, q] orientation.
  A ones-column appended to V gives the softmax denominator as column 64
  of each AV psum region -- no separate denominator matmuls.
- PSUM accumulations within one bank must be sequential (interleaving
  corrupts earlier regions), so heads are processed one at a time: head
  h's scores+exp stream in block h while head h-1's AV regions run
  region-major (kb innermost), packed 7-per-bank into 2 rolling psum
  banks one block behind.
- scores psum pool is 3-deep so the scores->exp->free chain never
  throttles the slot cadence; projections run as compact units through
  the same rotation.
- normalization is a per-partition DVE reciprocal + tensor_scalar
  multiply (q on partitions); normalized [q, 128] pair tiles go back to
  [inner, q] via DMA xbar transposes (zero PE cost), issue alternating
  between the vector and sync queues.
- input loads are tiered: critical path (cT, xT, wq0) serialized on the
  sync ring, wv on the scalar ring, bulk wk/wq chunks on the gpsimd
  ring behind a gate op that waits for cT so they cannot steal DMA
  bandwidth from the critical path.
"""

import sys

for _p in ("/opt/trn_rl_repo", "/root/.axon_site/_ro/trn_rl_repo"):
    if _p not in sys.path:
        sys.path.append(_p)

import numpy as np

import concourse.bass as bass
import concourse.tile as tile
from concourse import bacc, mybir
from concourse.bass_utils import run_bass_kernel_spmd

F32 = mybir.dt.float32
BF16 = mybir.dt.bfloat16
EXP = mybir.ActivationFunctionType.Exp
ADD = mybir.AluOpType.add

P = 128
B, NQ_FULL, DQ = 4, 2048, 1024
NK, DC = 1024, 768
H, DH = 16, 64
INNER = H * DH  # 1024
NT = 1024  # local queries per core
N_CORES = 8

KQ = DQ // P      # 8
KC = DC // P      # 6
KI = INNER // P   # 8
TB = NT // P      # 8 query tiles
KB = NK // P      # 8 kpos chunks
QB = NT // P      # 8 q-blocks for AV
HP = H // 2       # 8 head pairs
SCALE = 1.0 / np.sqrt(DH)


def build(dbg=False):
    nc = bacc.Bacc("TRN2", target_bir_lowering=False, debug=False,
                   enable_asserts=False, num_devices=N_CORES)

    cT_d = nc.dram_tensor("cT", [DC, NK], BF16, kind="ExternalInput")
    xpk_d = nc.dram_tensor("xpk", [P, 2, KQ, 512], BF16, kind="ExternalInput")
    wqpk_d = nc.dram_tensor("wqpk", [P, KI, KQ, P], BF16,
                            kind="ExternalInput")
    wkpk_d = nc.dram_tensor("wkpk", [P, KI, KC, P], BF16,
                            kind="ExternalInput")
    wvpk_d = nc.dram_tensor("wvpk", [P, 2, KC, 512], BF16,
                            kind="ExternalInput")
    wo_d = nc.dram_tensor("wo", [INNER, DQ], BF16, kind="ExternalInput")
    bo_d = nc.dram_tensor("bo", [DQ], BF16, kind="ExternalInput")
    out_d = nc.dram_tensor("out", [NT, DQ], BF16, kind="ExternalOutput")
    if dbg:
        dqT = nc.dram_tensor("dqT", [P, KI, NT], F32, kind="ExternalOutput")
        dkT = nc.dram_tensor("dkT", [P, KI, NK], F32, kind="ExternalOutput")
        dvA = nc.dram_tensor("dvA", [P, KB, H, DH + 1], F32,
                             kind="ExternalOutput")
        dattnT = nc.dram_tensor("dattnT", [P, KI, NT], F32,
                                kind="ExternalOutput")

    with tile.TileContext(nc) as tc:
        with (
            tc.tile_pool(name="persist", bufs=1) as persist,
            tc.tile_pool(name="psS", bufs=3, space="PSUM") as psS,
            tc.tile_pool(name="psAV", bufs=2, space="PSUM") as psAV,
            tc.tile_pool(name="etp", bufs=18) as etp,
            tc.tile_pool(name="aqp", bufs=20) as aqp,
            tc.tile_pool(name="recp", bufs=8) as recp,
            tc.tile_pool(name="outp", bufs=2) as outp,
        ):
            # persistent SBUF tensors
            cT = persist.tile([P, KC, NK], BF16)          # [dc, kpos]
            xT = persist.tile([P, 2, KQ, 512], BF16)      # [dq, (half,kc,q)]
            wq_b = persist.tile([P, KI, KQ, P], BF16)
            wk_b = persist.tile([P, KI, KC, P], BF16)
            wv_b = persist.tile([P, 2, KC, 512], BF16)
            wo_b = persist.tile([P, KI, DQ], BF16)
            bo_sb = persist.tile([1, DQ], BF16)
            ones_b = persist.tile([1, P], BF16)
            bias_b = persist.tile([P, DQ], BF16)          # bo bcast over parts
            qT = persist.tile([P, KI, NT], BF16)          # [inner, q]
            kT = persist.tile([P, KI, NK], BF16)          # [inner, kpos]
            vA = persist.tile([P, KB, H, DH + 1], BF16)   # [kpos,(h, d|1)]
            attnT = persist.tile([P, KI, NT], BF16)       # normalized attn^T

            # ---------------- input loads (tiered, consumer order) ------
            # The scheduler keeps emission order among ready DMAs per queue,
            # so the critical path (cT -> xT/wq0) leads all three DMA-capable
            # rings; bulk wk/wq chunks trail on the gpsimd ring.
            cT3 = cT_d.ap().rearrange("(o p) m -> p o m", p=P)
            wo4 = wo_d.ap().rearrange("(o p) m -> p o m", p=P)
            nc.sync.dma_start(bo_sb[:], bo_d.ap()[None, :])
            nc.sync.dma_start(cT[:, 0:2], cT3[:, 0:2])
            nc.sync.dma_start(xT[:, 0, 0:4], xpk_d.ap()[:, 0, 0:4])
            nc.sync.dma_start(xT[:, 1, 0:4], xpk_d.ap()[:, 1, 0:4])
            nc.scalar.dma_start(cT[:, 2:4], cT3[:, 2:4])
            nc.scalar.dma_start(xT[:, 0, 4:8], xpk_d.ap()[:, 0, 4:8])
            nc.scalar.dma_start(xT[:, 1, 4:8], xpk_d.ap()[:, 1, 4:8])
            nc.scalar.dma_start(wv_b[:, 0], wvpk_d.ap()[:, 0])
            nc.scalar.dma_start(wv_b[:, 1], wvpk_d.ap()[:, 1])
            nc.gpsimd.memset(vA[:, :, :, DH:DH + 1], 1.0)
            nc.gpsimd.memset(ones_b[:], 1.0)
            nc.gpsimd.dma_start(wk_b[:, 0], wkpk_d.ap()[:, 0])
            nc.gpsimd.dma_start(cT[:, 4:6], cT3[:, 4:6])
            nc.gpsimd.dma_start(wq_b[:, 0], wqpk_d.ap()[:, 0])
            nc.gpsimd.dma_start(wk_b[:, 1], wkpk_d.ap()[:, 1])
            nc.gpsimd.dma_start(wk_b[:, 2], wkpk_d.ap()[:, 2])
            nc.gpsimd.dma_start(wq_b[:, 1], wqpk_d.ap()[:, 1])
            for ko in (3, 4, 5, 6, 7):
                nc.gpsimd.dma_start(wk_b[:, ko], wkpk_d.ap()[:, ko])
                nc.gpsimd.dma_start(wq_b[:, ko - 1], wqpk_d.ap()[:, ko - 1])
            nc.gpsimd.dma_start(wq_b[:, 7], wqpk_d.ap()[:, 7])
            nc.gpsimd.dma_start(wo_b[:], wo4)

            # ---------------- projection units (psS rotation) -----------
            def kproj(ko):
                ps = psS.tile([P, NT], F32, tag="big", name=f"kp{ko}")
                # ko=0 runs during the input loads: accumulate in the order
                # the cT thirds land (scalar, sync, gpsimd rings)
                kcs = (2, 3, 0, 1, 4, 5) if ko == 0 else tuple(range(KC))
                for n0 in (0, 512):
                    for i, kc in enumerate(kcs):
                        nc.tensor.matmul(
                            ps[:, n0:n0 + 512],
                            wk_b[:, ko, kc, :],
                            cT[:, kc, n0:n0 + 512],
                            start=(i == 0), stop=(i == KC - 1))
                nc.vector.tensor_copy(kT[:, ko, :], ps[:])

            def qproj(ko):
                ps = psS.tile([P, NT], F32, tag="big", name=f"qp{ko}")
                if ko == 0:
                    # bias_b broadcast rides in this psum tile first
                    for n0 in (0, 512):
                        nc.tensor.matmul(ps[:, n0:n0 + 512], ones_b[0:1, :],
                                         bo_sb[0:1, n0:n0 + 512],
                                         start=True, stop=True)
                    nc.vector.tensor_copy(bias_b[:], ps[:])
                for hf in (0, 1):
                    n0 = hf * 512
                    for kc in range(KQ):
                        nc.tensor.matmul(
                            ps[:, n0:n0 + 512],
                            wq_b[:, ko, kc, :],
                            xT[:, hf, kc, :],
                            start=(kc == 0), stop=(kc == KQ - 1))
                    nc.vector.tensor_copy(qT[:, ko, n0:n0 + 512],
                                          ps[:, n0:n0 + 512])

            def vproj(mt, half):
                ps = psS.tile([P, NT], F32, tag="big", name=f"vp{mt}_{half}")
                for kc in range(KC):
                    nc.tensor.matmul(
                        ps[:, 0:512],
                        cT[:, kc, mt * P:(mt + 1) * P],
                        wv_b[:, half, kc, :],
                        start=(kc == 0), stop=(kc == KC - 1))
                h0 = half * 8
                nc.vector.tensor_copy(
                    vA[:, mt, h0:h0 + 8, 0:DH],
                    ps[:, 0:512].rearrange("p (h d) -> p h d", d=DH))

            # ---------------- attention ----------------
            # AV regions packed 7-per-bank into rolling psum banks.
            av_banks = {}

            def av_region(g):
                b, off = divmod(g, 7)
                if b not in av_banks:
                    av_banks[b] = psAV.tile([P, 512], F32, tag="av",
                                            name=f"avb{b}")
                return av_banks[b][:, off * 65:off * 65 + 65]

            ets = {}       # (h, kb) -> exp tile
            aqs = {}       # (hp, qb) -> normalized pair tile
            pending_tp = []  # transposes deferred a block so sync never
                             # holds its SEQ waiting on fresh aq tiles

            def flush_tp(n=1):
                for _ in range(n):
                    if not pending_tp:
                        return
                    hp, qb = pending_tp.pop(0)
                    nc.sync.dma_start_transpose(
                        attnT[:, hp, qb * P:(qb + 1) * P],
                        aqs.pop((hp, qb))[:])

            def emit_av_region(h, qb):
                # region-major: one full kb accumulation, sequential in bank
                reg = av_region(h * QB + qb)
                for kb in range(KB):
                    nc.tensor.matmul(
                        reg,
                        ets[(h, kb)][:, qb * P:(qb + 1) * P],
                        vA[:, kb, h, :],
                        start=(kb == 0), stop=(kb == KB - 1))

            def emit_norm(h, qb):
                hp, hl = divmod(h, 2)
                reg = av_region(h * QB + qb)
                rec = recp.tile([P, 1], F32, tag="rec", name=f"rec{h}_{qb}")
                if hl == 0:
                    aqs[(hp, qb)] = aqp.tile([P, P], BF16, tag="aq",
                                             name=f"aq{hp}_{qb}")
                aq = aqs[(hp, qb)]
                nc.vector.reciprocal_approx_fast(rec[:], reg[:, DH:DH + 1])
                nc.vector.tensor_scalar_mul(
                    aq[:, hl * DH:(hl + 1) * DH], reg[:, 0:DH], rec[:])
                if hl == 1:
                    pending_tp.append((hp, qb))

            def head_block(h, extra):
                hp, hl = divmod(h, 2)
                base = hl * DH
                for kb in range(KB):
                    flush_tp(1)
                    ps = psS.tile([P, NT], F32, tag="big", name=f"sc{h}_{kb}")
                    for n0 in (0, 512):
                        nc.tensor.matmul(
                            ps[:, n0:n0 + 512],
                            kT[base:base + DH, hp, kb * P:(kb + 1) * P],
                            qT[base:base + DH, hp, n0:n0 + 512],
                            start=True, stop=True)
                    # cluster all AV regions in one stream to cut PE
                    # switches; region-major order within banks preserved
                    if h > 0 and kb == 1:
                        for qb in range(QB):
                            emit_av_region(h - 1, qb)
                    for fn in extra[kb]:
                        fn()
                    et = etp.tile([P, NT], BF16, tag="exp", name=f"et{h}_{kb}")
                    nc.scalar.activation(et[:], ps[:], EXP, scale=float(SCALE))
                    ets[(h, kb)] = et
                    if h > 0 and kb == 1:
                        for qb in range(QB):
                            emit_norm(h - 1, qb)

            # ---------------- out projection unit ------------------------
            out3 = out_d.ap().rearrange("(t p) d -> p t d", p=P)
            out_ps = {}

            def out_unit(mt, kcs, finish):
                if mt not in out_ps:
                    out_ps[mt] = psS.tile([P, NT], F32, tag="big",
                                          name=f"op{mt}")
                ps = out_ps[mt]
                # kc-outer; n0 banks see sequential accumulation streams
                for kc in kcs:
                    for n0 in (0, 512):
                        nc.tensor.matmul(
                            ps[:, n0:n0 + 512],
                            attnT[:, kc, mt * P:(mt + 1) * P],
                            wo_b[:, kc, n0:n0 + 512],
                            start=(kc == 0), stop=(finish and kc == KI - 1))
                if finish:
                    ot = outp.tile([P, DQ], BF16, tag="out", name=f"ot{mt}")
                    nc.vector.tensor_tensor(ot[:], ps[:], bias_b[:], ADD)
                    eng = nc.sync if mt % 2 == 0 else nc.scalar
                    eng.dma_start(out3[:, mt], ot[:])

            # ---------------- schedule ----------------
            # interleave the first k/q projections so the PE fills the
            # wait for the last cT third / xT halves with useful work
            kp0 = psS.tile([P, NT], F32, tag="big", name="kp0")
            qp0 = psS.tile([P, NT], F32, tag="big", name="qp0")
            for n0 in (0, 512):
                for i, kc in enumerate((2, 3, 0, 1)):
                    nc.tensor.matmul(kp0[:, n0:n0 + 512], wk_b[:, 0, kc, :],
                                     cT[:, kc, n0:n0 + 512],
                                     start=(i == 0), stop=False)
            bp0 = psS.tile([P, NT], F32, tag="big", name="bp0")
            for n0 in (0, 512):
                nc.tensor.matmul(bp0[:, n0:n0 + 512], ones_b[0:1, :],
                                 bo_sb[0:1, n0:n0 + 512],
                                 start=True, stop=True)
            nc.vector.tensor_copy(bias_b[:], bp0[:])
            for kc in range(KQ):
                nc.tensor.matmul(qp0[:, 0:512], wq_b[:, 0, kc, :],
                                 xT[:, 0, kc, :],
                                 start=(kc == 0), stop=(kc == KQ - 1))
            nc.vector.tensor_copy(qT[:, 0, 0:512], qp0[:, 0:512])
            for n0 in (0, 512):
                for i, kc in enumerate((4, 5)):
                    nc.tensor.matmul(kp0[:, n0:n0 + 512], wk_b[:, 0, kc, :],
                                     cT[:, kc, n0:n0 + 512],
                                     start=False, stop=(i == 1))
            nc.vector.tensor_copy(kT[:, 0, :], kp0[:])
            for kc in range(KQ):
                nc.tensor.matmul(qp0[:, 512:1024], wq_b[:, 0, kc, :],
                                 xT[:, 1, kc, :],
                                 start=(kc == 0), stop=(kc == KQ - 1))
            nc.vector.tensor_copy(qT[:, 0, 512:1024], qp0[:, 512:1024])
            for h in range(H):
                hp, hl = divmod(h, 2)
                extra = [[] for _ in range(KB)]
                if h == 0:
                    for mt in range(KB):
                        extra[mt].append(lambda mt=mt: vproj(mt, 0))
                    extra[4].append(lambda: kproj(1))
                if 1 <= h <= 8:
                    mt = h - 1
                    extra[6 if h < 8 else 4].append(
                        lambda mt=mt: vproj(mt, 1))
                # balanced projection placement: kproj on even blocks,
                # qproj on odd blocks, away from the slot-1 AV cluster
                if hl == 0 and 0 < hp < HP - 1:
                    extra[4].append(lambda ko=hp + 1: kproj(ko))
                if hl == 1 and hp < HP - 1:
                    extra[4].append(lambda ko=hp + 1: qproj(ko))
                if h == H - 1:
                    extra[3].append(lambda: out_unit(0, range(KI - 1), False))
                head_block(h, extra)

            # ---------------- tail: last head's AV + out projection -----
            for mt in range(TB):
                emit_av_region(H - 1, mt)
                emit_norm(H - 1, mt)
                flush_tp(2)
            flush_tp(len(pending_tp))
            out_unit(0, [KI - 1], True)
            for mt in range(1, TB):
                out_unit(mt, range(KI), True)

            if dbg:
                nc.gpsimd.dma_start(dqT.ap(), qT[:])
                nc.gpsimd.dma_start(dkT.ap(), kT[:])
                nc.gpsimd.dma_start(dvA.ap(), vA[:])
                nc.gpsimd.dma_start(dattnT.ap(), attnT[:])

    nc.compile()
    return nc


_NC_CACHE = None


def _make_in_maps(inputs):
    import ml_dtypes
    bf = ml_dtypes.bfloat16
    x = np.asarray(inputs["x"], dtype=np.float32).astype(bf)
    context = np.asarray(inputs["context"], dtype=np.float32).astype(bf)
    wq = np.asarray(inputs["Wq"], np.float32).astype(bf)
    wk = np.asarray(inputs["Wk"], np.float32).astype(bf)
    wv = np.asarray(inputs["Wv"], np.float32).astype(bf)
    shared = {
        # [dq_chunk p, ko, kc, j]: per-(p, ko) contiguous 1536/2048B runs
        "wqpk": np.ascontiguousarray(
            wq.reshape(KQ, P, KI, P).transpose(1, 2, 0, 3)),
        "wkpk": np.ascontiguousarray(
            wk.reshape(KC, P, KI, P).transpose(1, 2, 0, 3)),
        "wvpk": np.ascontiguousarray(
            wv.reshape(KC, P, 2, 512).transpose(1, 2, 0, 3)),
        "wo": np.ascontiguousarray(np.asarray(inputs["Wo"], np.float32)
                                   .astype(bf)),
        "bo": np.ascontiguousarray(np.asarray(inputs["bo"], np.float32)
                                   .astype(bf)),
    }
    in_maps = []
    for c in range(N_CORES):
        b, s = divmod(c, 2)
        xTh = np.ascontiguousarray(x[b, s * NT:(s + 1) * NT, :].T)  # [dq, q]
        in_maps.append({
            "xpk": np.ascontiguousarray(
                xTh.reshape(KQ, P, 2, 512).transpose(1, 2, 0, 3)),
            "cT": np.ascontiguousarray(context[b].T),
            **shared,
        })
    return in_maps


def kernel(x, context, Wq, Wk, Wv, Wo, bo):
    global _NC_CACHE
    if _NC_CACHE is None:
        _NC_CACHE = build()
    nc = _NC_CACHE

    in_maps = _make_in_maps(dict(x=x, context=context, Wq=Wq, Wk=Wk, Wv=Wv,
                                 Wo=Wo, bo=bo))
    res = run_bass_kernel_spmd(nc, in_maps, core_ids=list(range(N_CORES)))
    out = np.empty((B, NQ_FULL, DQ), dtype=np.float32)
    for c in range(N_CORES):
        b, s = divmod(c, 2)
        out[b, s * NT:(s + 1) * NT, :] = res.results[c]["out"].astype(
            np.float32)
    return out


# revision 17
# speedup vs baseline: 1.3065x; 1.0088x over previous
"""CrossAttention kernel for 8 TRN2 NeuronCores.

Sharding: 8 cores = 4 batches x 2 query-halves (zero communication).
Each core computes all 16 heads for its 1024 queries.

v8 structure:
- AV computed in [q, d] orientation (lhsT = exp-scores tile, rhs = V):
  66.5k streamed columns instead of 131k for the [d, q] orientation.
  A ones-column appended to V gives the softmax denominator as column 64
  of each AV psum region -- no separate denominator matmuls.
- PSUM accumulations within one bank must be sequential (interleaving
  corrupts earlier regions), so heads are processed one at a time: head
  h's scores+exp stream in block h while head h-1's AV regions run
  region-major (kb innermost), packed 7-per-bank into 2 rolling psum
  banks one block behind.
- scores psum pool is 3-deep so the scores->exp->free chain never
  throttles the slot cadence; projections run as compact units through
  the same rotation.
- normalization is a per-partition DVE reciprocal + tensor_scalar
  multiply (q on partitions); normalized [q, 128] pair tiles go back to
  [inner, q] via DMA xbar transposes (zero PE cost), issue alternating
  between the vector and sync queues.
- input loads are tiered: critical path (cT, xT, wq0) serialized on the
  sync ring, wv on the scalar ring, bulk wk/wq chunks on the gpsimd
  ring behind a gate op that waits for cT so they cannot steal DMA
  bandwidth from the critical path.
"""

import sys

for _p in ("/opt/trn_rl_repo", "/root/.axon_site/_ro/trn_rl_repo"):
    if _p not in sys.path:
        sys.path.append(_p)

import numpy as np

import concourse.bass as bass
import concourse.tile as tile
from concourse import bacc, mybir
from concourse.bass_utils import run_bass_kernel_spmd

F32 = mybir.dt.float32
BF16 = mybir.dt.bfloat16
EXP = mybir.ActivationFunctionType.Exp
ADD = mybir.AluOpType.add

P = 128
B, NQ_FULL, DQ = 4, 2048, 1024
NK, DC = 1024, 768
H, DH = 16, 64
INNER = H * DH  # 1024
NT = 1024  # local queries per core
N_CORES = 8

KQ = DQ // P      # 8
KC = DC // P      # 6
KI = INNER // P   # 8
TB = NT // P      # 8 query tiles
KB = NK // P      # 8 kpos chunks
QB = NT // P      # 8 q-blocks for AV
HP = H // 2       # 8 head pairs
SCALE = 1.0 / np.sqrt(DH)


def build(dbg=False):
    nc = bacc.Bacc("TRN2", target_bir_lowering=False, debug=False,
                   enable_asserts=False, num_devices=N_CORES)

    cT_d = nc.dram_tensor("cT", [DC, NK], BF16, kind="ExternalInput")
    xpk_d = nc.dram_tensor("xpk", [P, 2, KQ, 512], BF16, kind="ExternalInput")
    wqpk_d = nc.dram_tensor("wqpk", [P, KI, KQ, P], BF16,
                            kind="ExternalInput")
    wkpk_d = nc.dram_tensor("wkpk", [P, KI, KC, P], BF16,
                            kind="ExternalInput")
    wvpk_d = nc.dram_tensor("wvpk", [P, 2, KC, 512], BF16,
                            kind="ExternalInput")
    wo_d = nc.dram_tensor("wo", [INNER, DQ], BF16, kind="ExternalInput")
    bo_d = nc.dram_tensor("bo", [DQ], BF16, kind="ExternalInput")
    out_d = nc.dram_tensor("out", [NT, DQ], BF16, kind="ExternalOutput")
    if dbg:
        dqT = nc.dram_tensor("dqT", [P, KI, NT], F32, kind="ExternalOutput")
        dkT = nc.dram_tensor("dkT", [P, KI, NK], F32, kind="ExternalOutput")
        dvA = nc.dram_tensor("dvA", [P, KB, H, DH + 1], F32,
                             kind="ExternalOutput")
        dattnT = nc.dram_tensor("dattnT", [P, KI, NT], F32,
                                kind="ExternalOutput")

    with tile.TileContext(nc) as tc:
        with (
            tc.tile_pool(name="persist", bufs=1) as persist,
            tc.tile_pool(name="psS", bufs=3, space="PSUM") as psS,
            tc.tile_pool(name="psAV", bufs=2, space="PSUM") as psAV,
            tc.tile_pool(name="etp", bufs=18) as etp,
            tc.tile_pool(name="aqp", bufs=20) as aqp,
            tc.tile_pool(name="recp", bufs=8) as recp,
            tc.tile_pool(name="outp", bufs=2) as outp,
        ):
            # persistent SBUF tensors
            cT = persist.tile([P, KC, NK], BF16)          # [dc, kpos]
            xT = persist.tile([P, 2, KQ, 512], BF16)      # [dq, (half,kc,q)]
            wq_b = persist.tile([P, KI, KQ, P], BF16)
            wk_b = persist.tile([P, KI, KC, P], BF16)
            wv_b = persist.tile([P, 2, KC, 512], BF16)
            wo_b = persist.tile([P, KI, DQ], BF16)
            bo_sb = persist.tile([1, DQ], BF16)
            ones_b = persist.tile([1, P], BF16)
            bias_b = persist.tile([P, DQ], BF16)          # bo bcast over parts
            qT = persist.tile([P, KI, NT], BF16)          # [inner, q]
            kT = persist.tile([P, KI, NK], BF16)          # [inner, kpos]
            vA = persist.tile([P, KB, H, DH + 1], BF16)   # [kpos,(h, d|1)]
            attnT = persist.tile([P, KI, NT], BF16)       # normalized attn^T

            # ---------------- input loads (tiered, consumer order) ------
            # The scheduler keeps emission order among ready DMAs per queue,
            # so the critical path (cT -> xT/wq0) leads all three DMA-capable
            # rings; bulk wk/wq chunks trail on the gpsimd ring.
            cT3 = cT_d.ap().rearrange("(o p) m -> p o m", p=P)
            wo4 = wo_d.ap().rearrange("(o p) m -> p o m", p=P)
            nc.sync.dma_start(bo_sb[:], bo_d.ap()[None, :])
            nc.sync.dma_start(cT[:, 0:2], cT3[:, 0:2])
            nc.sync.dma_start(xT[:, 0, 0:4], xpk_d.ap()[:, 0, 0:4])
            nc.scalar.dma_start(cT[:, 2:4], cT3[:, 2:4])
            nc.scalar.dma_start(xT[:, 0, 4:8], xpk_d.ap()[:, 0, 4:8])
            nc.scalar.dma_start(xT[:, 1, 4:8], xpk_d.ap()[:, 1, 4:8])
            nc.scalar.dma_start(wv_b[:, 0], wvpk_d.ap()[:, 0])
            nc.scalar.dma_start(wv_b[:, 1], wvpk_d.ap()[:, 1])
            nc.gpsimd.memset(vA[:, :, :, DH:DH + 1], 1.0)
            nc.gpsimd.memset(ones_b[:], 1.0)
            nc.gpsimd.dma_start(wk_b[:, 0], wkpk_d.ap()[:, 0])
            nc.gpsimd.dma_start(cT[:, 4:6], cT3[:, 4:6])
            nc.gpsimd.dma_start(wq_b[:, 0], wqpk_d.ap()[:, 0])
            nc.gpsimd.dma_start(xT[:, 1, 0:4], xpk_d.ap()[:, 1, 0:4])
            nc.gpsimd.dma_start(wk_b[:, 1], wkpk_d.ap()[:, 1])
            nc.gpsimd.dma_start(wk_b[:, 2], wkpk_d.ap()[:, 2])
            nc.gpsimd.dma_start(wq_b[:, 1], wqpk_d.ap()[:, 1])
            for ko in (3, 4, 5, 6, 7):
                nc.gpsimd.dma_start(wk_b[:, ko], wkpk_d.ap()[:, ko])
                nc.gpsimd.dma_start(wq_b[:, ko - 1], wqpk_d.ap()[:, ko - 1])
            nc.gpsimd.dma_start(wq_b[:, 7], wqpk_d.ap()[:, 7])
            nc.gpsimd.dma_start(wo_b[:], wo4)

            # ---------------- projection units (psS rotation) -----------
            def kproj(ko):
                ps = psS.tile([P, NT], F32, tag="big", name=f"kp{ko}")
                # ko=0 runs during the input loads: accumulate in the order
                # the cT thirds land (scalar, sync, gpsimd rings)
                kcs = (2, 3, 0, 1, 4, 5) if ko == 0 else tuple(range(KC))
                for n0 in (0, 512):
                    for i, kc in enumerate(kcs):
                        nc.tensor.matmul(
                            ps[:, n0:n0 + 512],
                            wk_b[:, ko, kc, :],
                            cT[:, kc, n0:n0 + 512],
                            start=(i == 0), stop=(i == KC - 1))
                nc.vector.tensor_copy(kT[:, ko, :], ps[:])

            def qproj(ko):
                ps = psS.tile([P, NT], F32, tag="big", name=f"qp{ko}")
                if ko == 0:
                    # bias_b broadcast rides in this psum tile first
                    for n0 in (0, 512):
                        nc.tensor.matmul(ps[:, n0:n0 + 512], ones_b[0:1, :],
                                         bo_sb[0:1, n0:n0 + 512],
                                         start=True, stop=True)
                    nc.vector.tensor_copy(bias_b[:], ps[:])
                for hf in (0, 1):
                    n0 = hf * 512
                    for kc in range(KQ):
                        nc.tensor.matmul(
                            ps[:, n0:n0 + 512],
                            wq_b[:, ko, kc, :],
                            xT[:, hf, kc, :],
                            start=(kc == 0), stop=(kc == KQ - 1))
                    nc.vector.tensor_copy(qT[:, ko, n0:n0 + 512],
                                          ps[:, n0:n0 + 512])

            def vproj(mt, half):
                ps = psS.tile([P, NT], F32, tag="big", name=f"vp{mt}_{half}")
                for kc in range(KC):
                    nc.tensor.matmul(
                        ps[:, 0:512],
                        cT[:, kc, mt * P:(mt + 1) * P],
                        wv_b[:, half, kc, :],
                        start=(kc == 0), stop=(kc == KC - 1))
                h0 = half * 8
                nc.vector.tensor_copy(
                    vA[:, mt, h0:h0 + 8, 0:DH],
                    ps[:, 0:512].rearrange("p (h d) -> p h d", d=DH))

            # ---------------- attention ----------------
            # AV regions packed 7-per-bank into rolling psum banks.
            av_banks = {}

            def av_region(g):
                b, off = divmod(g, 7)
                if b not in av_banks:
                    av_banks[b] = psAV.tile([P, 512], F32, tag="av",
                                            name=f"avb{b}")
                return av_banks[b][:, off * 65:off * 65 + 65]

            ets = {}       # (h, kb) -> exp tile
            aqs = {}       # (hp, qb) -> normalized pair tile
            pending_tp = []  # transposes deferred a block so sync never
                             # holds its SEQ waiting on fresh aq tiles

            def flush_tp(n=1):
                for _ in range(n):
                    if not pending_tp:
                        return
                    hp, qb = pending_tp.pop(0)
                    nc.sync.dma_start_transpose(
                        attnT[:, hp, qb * P:(qb + 1) * P],
                        aqs.pop((hp, qb))[:])

            def emit_av_region(h, qb):
                # region-major: one full kb accumulation, sequential in bank
                reg = av_region(h * QB + qb)
                for kb in range(KB):
                    nc.tensor.matmul(
                        reg,
                        ets[(h, kb)][:, qb * P:(qb + 1) * P],
                        vA[:, kb, h, :],
                        start=(kb == 0), stop=(kb == KB - 1))

            def emit_norm(h, qb):
                hp, hl = divmod(h, 2)
                reg = av_region(h * QB + qb)
                rec = recp.tile([P, 1], F32, tag="rec", name=f"rec{h}_{qb}")
                if hl == 0:
                    aqs[(hp, qb)] = aqp.tile([P, P], BF16, tag="aq",
                                             name=f"aq{hp}_{qb}")
                aq = aqs[(hp, qb)]
                nc.vector.reciprocal_approx_fast(rec[:], reg[:, DH:DH + 1])
                nc.vector.tensor_scalar_mul(
                    aq[:, hl * DH:(hl + 1) * DH], reg[:, 0:DH], rec[:])
                if hl == 1:
                    pending_tp.append((hp, qb))

            def head_block(h, extra):
                hp, hl = divmod(h, 2)
                base = hl * DH
                for kb in range(KB):
                    flush_tp(1)
                    ps = psS.tile([P, NT], F32, tag="big", name=f"sc{h}_{kb}")
                    for n0 in (0, 512):
                        nc.tensor.matmul(
                            ps[:, n0:n0 + 512],
                            kT[base:base + DH, hp, kb * P:(kb + 1) * P],
                            qT[base:base + DH, hp, n0:n0 + 512],
                            start=True, stop=True)
                    # cluster all AV regions in one stream to cut PE
                    # switches; region-major order within banks preserved
                    if h > 0 and kb == 2:
                        for qb in range(QB):
                            emit_av_region(h - 1, qb)
                    for fn in extra[kb]:
                        fn()
                    et = etp.tile([P, NT], BF16, tag="exp", name=f"et{h}_{kb}")
                    nc.scalar.activation(et[:], ps[:], EXP, scale=float(SCALE))
                    ets[(h, kb)] = et
                    if h > 0 and kb == 2:
                        for qb in range(QB):
                            emit_norm(h - 1, qb)

            # ---------------- out projection unit ------------------------
            out3 = out_d.ap().rearrange("(t p) d -> p t d", p=P)
            out_ps = {}

            def out_unit(mt, kcs, finish):
                if mt not in out_ps:
                    out_ps[mt] = psS.tile([P, NT], F32, tag="big",
                                          name=f"op{mt}")
                ps = out_ps[mt]
                # kc-outer; n0 banks see sequential accumulation streams
                for kc in kcs:
                    for n0 in (0, 512):
                        nc.tensor.matmul(
                            ps[:, n0:n0 + 512],
                            attnT[:, kc, mt * P:(mt + 1) * P],
                            wo_b[:, kc, n0:n0 + 512],
                            start=(kc == 0), stop=(finish and kc == KI - 1))
                if finish:
                    ot = outp.tile([P, DQ], BF16, tag="out", name=f"ot{mt}")
                    nc.vector.tensor_tensor(ot[:], ps[:], bias_b[:], ADD)
                    eng = nc.sync if mt % 2 == 0 else nc.scalar
                    eng.dma_start(out3[:, mt], ot[:])

            # ---------------- schedule ----------------
            # interleave the first k/q projections so the PE fills the
            # wait for the last cT third / xT halves with useful work
            kp0 = psS.tile([P, NT], F32, tag="big", name="kp0")
            qp0 = psS.tile([P, NT], F32, tag="big", name="qp0")
            for n0 in (0, 512):
                for i, kc in enumerate((2, 3, 0, 1)):
                    nc.tensor.matmul(kp0[:, n0:n0 + 512], wk_b[:, 0, kc, :],
                                     cT[:, kc, n0:n0 + 512],
                                     start=(i == 0), stop=False)
            bp0 = psS.tile([P, NT], F32, tag="big", name="bp0")
            for n0 in (0, 512):
                nc.tensor.matmul(bp0[:, n0:n0 + 512], ones_b[0:1, :],
                                 bo_sb[0:1, n0:n0 + 512],
                                 start=True, stop=True)
            nc.vector.tensor_copy(bias_b[:], bp0[:])
            for kc in range(KQ):
                nc.tensor.matmul(qp0[:, 0:512], wq_b[:, 0, kc, :],
                                 xT[:, 0, kc, :],
                                 start=(kc == 0), stop=(kc == KQ - 1))
            nc.vector.tensor_copy(qT[:, 0, 0:512], qp0[:, 0:512])
            for n0 in (0, 512):
                for i, kc in enumerate((4, 5)):
                    nc.tensor.matmul(kp0[:, n0:n0 + 512], wk_b[:, 0, kc, :],
                                     cT[:, kc, n0:n0 + 512],
                                     start=False, stop=(i == 1))
            nc.vector.tensor_copy(kT[:, 0, :], kp0[:])
            for kc in range(KQ):
                nc.tensor.matmul(qp0[:, 512:1024], wq_b[:, 0, kc, :],
                                 xT[:, 1, kc, :],
                                 start=(kc == 0), stop=(kc == KQ - 1))
            nc.vector.tensor_copy(qT[:, 0, 512:1024], qp0[:, 512:1024])
            for h in range(H):
                hp, hl = divmod(h, 2)
                extra = [[] for _ in range(KB)]
                if h == 0:
                    for mt in range(KB):
                        extra[mt].append(lambda mt=mt: vproj(mt, 0))
                    extra[6].append(lambda: kproj(1))
                if 1 <= h <= 8:
                    mt = h - 1
                    extra[6 if h < 8 else 4].append(
                        lambda mt=mt: vproj(mt, 1))
                # balanced projection placement: kproj on even blocks,
                # qproj on odd blocks, away from the slot-1 AV cluster
                if hl == 0 and 0 < hp < HP - 1:
                    extra[4].append(lambda ko=hp + 1: kproj(ko))
                if hl == 1 and hp < HP - 1:
                    extra[4].append(lambda ko=hp + 1: qproj(ko))
                if h == H - 1:
                    extra[3].append(lambda: out_unit(0, range(KI - 1), False))
                head_block(h, extra)

            # ---------------- tail: last head's AV + out projection -----
            for mt in range(TB):
                emit_av_region(H - 1, mt)
                emit_norm(H - 1, mt)
                flush_tp(2)
            flush_tp(len(pending_tp))
            out_unit(0, [KI - 1], True)
            for mt in range(1, TB):
                out_unit(mt, range(KI), True)

            if dbg:
                nc.gpsimd.dma_start(dqT.ap(), qT[:])
                nc.gpsimd.dma_start(dkT.ap(), kT[:])
                nc.gpsimd.dma_start(dvA.ap(), vA[:])
                nc.gpsimd.dma_start(dattnT.ap(), attnT[:])

    nc.compile()
    return nc


_NC_CACHE = None


def _make_in_maps(inputs):
    import ml_dtypes
    bf = ml_dtypes.bfloat16
    x = np.asarray(inputs["x"], dtype=np.float32).astype(bf)
    context = np.asarray(inputs["context"], dtype=np.float32).astype(bf)
    wq = np.asarray(inputs["Wq"], np.float32).astype(bf)
    wk = np.asarray(inputs["Wk"], np.float32).astype(bf)
    wv = np.asarray(inputs["Wv"], np.float32).astype(bf)
    shared = {
        # [dq_chunk p, ko, kc, j]: per-(p, ko) contiguous 1536/2048B runs
        "wqpk": np.ascontiguousarray(
            wq.reshape(KQ, P, KI, P).transpose(1, 2, 0, 3)),
        "wkpk": np.ascontiguousarray(
            wk.reshape(KC, P, KI, P).transpose(1, 2, 0, 3)),
        "wvpk": np.ascontiguousarray(
            wv.reshape(KC, P, 2, 512).transpose(1, 2, 0, 3)),
        "wo": np.ascontiguousarray(np.asarray(inputs["Wo"], np.float32)
                                   .astype(bf)),
        "bo": np.ascontiguousarray(np.asarray(inputs["bo"], np.float32)
                                   .astype(bf)),
    }
    in_maps = []
    for c in range(N_CORES):
        b, s = divmod(c, 2)
        xTh = np.ascontiguousarray(x[b, s * NT:(s + 1) * NT, :].T)  # [dq, q]
        in_maps.append({
            "xpk": np.ascontiguousarray(
                xTh.reshape(KQ, P, 2, 512).transpose(1, 2, 0, 3)),
            "cT": np.ascontiguousarray(context[b].T),
            **shared,
        })
    return in_maps


def kernel(x, context, Wq, Wk, Wv, Wo, bo):
    global _NC_CACHE
    if _NC_CACHE is None:
        _NC_CACHE = build()
    nc = _NC_CACHE

    in_maps = _make_in_maps(dict(x=x, context=context, Wq=Wq, Wk=Wk, Wv=Wv,
                                 Wo=Wo, bo=bo))
    res = run_bass_kernel_spmd(nc, in_maps, core_ids=list(range(N_CORES)))
    out = np.empty((B, NQ_FULL, DQ), dtype=np.float32)
    for c in range(N_CORES):
        b, s = divmod(c, 2)
        out[b, s * NT:(s + 1) * NT, :] = res.results[c]["out"].astype(
            np.float32)
    return out
